# revision 8
# baseline (speedup 1.0000x reference)
# Bidirectional 2-layer LSTM decoder on 8 Trainium2 NeuronCores.
#
# Decomposition: the network factors into independent (batch, direction)
# chains — directions only concatenate at the output, and layer 1 of a
# direction consumes only that direction's layer-0 output. So the 8 cores
# run one uniform SPMD program: core = (direction, batch-quarter), with
# the direction realized purely through per-core data (time-reversed x and
# that direction's weights).
#
# Per core (B_local=8, S=512, H=512):
#   GEMM0:  G0 = x @ Wih0^T + bias     (big matmul, written to DRAM)
#   REC0:   512-step LSTM recurrence, layer 0
#   GEMM1:  G1 = out0 @ Wih1^T + bias
#   REC1:   512-step recurrence, layer 1 -> out, final h/c
#
# Recurrence step (batch-major, gates column order [g|i|f|o]):
#   gates_psum  = I8.T @ G[t]          (identity matmul folds the
#                                       precomputed input term into PSUM)
#   gates_psum += h_{t-1} @ Whh^T      (h^T is the tiny stationary operand;
#                                       the weight matrix streams, which is
#                                       what the PE does at full rate)
#   ACT: tanh(g), sigmoid(i,f), sigmoid(o), tanh(c')
#   DVE: c' = sf*c + si*tg ; h' = so*tanh(c')
#   PE:  4x transpose h' -> h'^T       (stationary operand for step t+1)

import sys

import numpy as np

for _p in ("/opt/trn_rl_repo", "/root/.axon_site/_ro/trn_rl_repo"):
    if _p not in sys.path:
        sys.path.append(_p)

import concourse.bass as bass  # noqa: E402
import concourse.mybir as mybir  # noqa: E402
import concourse.tile as tile  # noqa: E402
from concourse import bacc  # noqa: E402

F32 = mybir.dt.float32
AF = mybir.ActivationFunctionType

H = 512
L = 2
B = 32
S = 512
D = 512
NCORES = 8
BL = B // (NCORES // 2)  # 8: batch rows per core (2 dirs x 4 quarters)
G4 = 4 * H  # 2048 gate columns

# reorder torch gate rows (i,f,g,o) -> (g,i,f,o) so tanh(g) input is ready
# first in the matmul stream and sigmoid(i,f) reads one contiguous slab
GATE_PERM = np.r_[2 * H : 3 * H, 0:H, H : 2 * H, 3 * H : 4 * H]
SL_G = slice(0, H)
SL_IF = slice(H, 3 * H)
SL_I = slice(H, 2 * H)
SL_F = slice(2 * H, 3 * H)
SL_O = slice(3 * H, 4 * H)


def _emit_gemm(nc, pools, S_, lhsT_src, rhs_sb, bias_sb, ones1, G_dram):
    """G_dram[m*128:(m+1)*128, :] = lhsT_m.T @ rhs (+ ones1.T @ bias row).

    lhsT_src(m) -> list of 4 [128,128] APs (K-chunks of the stationary
    operand for output row-tile m). rhs_sb is [128, 4, G4] in SBUF.
    """
    n_m = (S_ * BL) // 128
    for m in range(n_m):
        ps = pools["psum"].tile([128, G4], F32, tag="ps_main")
        lhsT = lhsT_src(m)
        for n in range(4):
            nc.tensor.matmul(
                ps[:, n * H : (n + 1) * H],
                ones1[:, m * 128 : (m + 1) * 128],
                bias_sb[:, n * H : (n + 1) * H],
                start=True,
                stop=False,
            )
        for k in range(4):
            for n in range(4):
                nc.tensor.matmul(
                    ps[:, n * H : (n + 1) * H],
                    lhsT[k],
                    rhs_sb[:, k, n * H : (n + 1) * H],
                    start=False,
                    stop=(k == 3),
                )
        gout = pools["gsb"].tile([128, G4], F32, tag="gsb")
        nc.scalar.copy(gout[:, 0 : G4 // 2], ps[:, 0 : G4 // 2])
        nc.vector.tensor_copy(gout[:, G4 // 2 : G4], ps[:, G4 // 2 : G4])
        nc.gpsimd.dma_start(G_dram[m * 128 : (m + 1) * 128, :], gout[:])


def _emit_recurrence(
    nc, pools, S_, layer, whh_sb, G_dram, ht_init, c_init, ident8,
    out0T_dram, out1_dram, hN_dram, cN_dram,
):
    """One 512-step LSTM chain. layer 0 stores h^T blocks (GEMM1 stationary);
    layer 1 stores the output sequence and both layers store final h/c."""
    blk = None
    ht_prev = None  # layer 1: rotating [128, 32] h^T tile
    prev_blk, prev_off = None, 0  # layer 0: block tile holding h^T_{t-1}
    c_prev = c_init
    for t in range(S_):
        g_sb = pools["gq"].tile([BL, G4], F32, tag="gq")
        nc.sync.dma_start(g_sb[:], G_dram[t * BL : (t + 1) * BL, :])

        gates = pools["psum"].tile([BL, G4], F32, tag="ps_main")
        for n in range(4):
            nc.tensor.matmul(
                gates[:, n * H : (n + 1) * H],
                ident8[:],
                g_sb[:, n * H : (n + 1) * H],
                start=True,
                stop=False,
            )
        for n in range(4):
            for k in range(4):
                if t == 0:
                    lhsT = ht_init[:, k * BL : (k + 1) * BL]
                elif layer == 0:
                    lhsT = prev_blk[:, k, prev_off * BL : (prev_off + 1) * BL]
                else:
                    lhsT = ht_prev[:, k * BL : (k + 1) * BL]
                nc.tensor.matmul(
                    gates[:, n * H : (n + 1) * H],
                    lhsT,
                    whh_sb[:, k, n * H : (n + 1) * H],
                    start=False,
                    stop=(k == 3),
                )

        tg = pools["act"].tile([BL, H], F32, tag="tg")
        nc.scalar.activation(tg[:], gates[:, SL_G], AF.Tanh)
        sif = pools["act"].tile([BL, 2 * H], F32, tag="sif")
        nc.scalar.activation(sif[:], gates[:, SL_IF], AF.Sigmoid)
        so = pools["act"].tile([BL, H], F32, tag="so")
        nc.scalar.activation(so[:], gates[:, SL_O], AF.Sigmoid)

        tmp1 = pools["dve"].tile([BL, H], F32, tag="tmp1")
        nc.vector.tensor_mul(tmp1[:], sif[:, H : 2 * H], c_prev[:])
        tmp2 = pools["dve"].tile([BL, H], F32, tag="tmp2")
        nc.vector.tensor_mul(tmp2[:], sif[:, 0:H], tg[:])
        c_new = pools["c"].tile([BL, H], F32, tag="c")
        nc.vector.tensor_add(c_new[:], tmp1[:], tmp2[:])
        tc_t = pools["act"].tile([BL, H], F32, tag="tc")
        nc.scalar.activation(tc_t[:], c_new[:], AF.Tanh)
        h_new = pools["h"].tile([BL, H], F32, tag="h")
        nc.vector.tensor_mul(h_new[:], so[:], tc_t[:])

        # h' -> h'^T (4 x [8,128] -> [128,8] PE transposes, one ACT copy out)
        tps = pools["tps"].tile([128, 4 * BL], F32, tag="tps")
        for k in range(4):
            nc.tensor.transpose(
                tps[:, k * BL : (k + 1) * BL],
                h_new[:, k * 128 : (k + 1) * 128],
                ident8[:],
            )
        if layer == 0:
            off = t % 16
            if off == 0:
                blk = pools["blk"].tile([128, 4, 16 * BL], F32, tag="blk")
            dst = blk[:, :, off * BL : (off + 1) * BL]
            nc.scalar.copy(dst, tps[:].rearrange("p (k b) -> p k b", b=BL))
            if off == 15:
                m = t // 16
                nblk = 128 * 4 * 16 * BL  # elems per 16-step h^T block
                nc.gpsimd.dma_start(
                    out0T_dram[m * nblk : (m + 1) * nblk]
                    .rearrange("(p k b) -> p k b", p=128, k=4),
                    blk[:],
                )
            prev_blk, prev_off = blk, off
        else:
            ht_new = pools["ht"].tile([128, 4 * BL], F32, tag="ht")
            nc.scalar.copy(ht_new[:], tps[:])
            ht_prev = ht_new
            nc.gpsimd.dma_start(out1_dram[t * BL : (t + 1) * BL, :], h_new[:])

        if t == S_ - 1:
            nc.gpsimd.dma_start(hN_dram[layer], h_new[:])
            nc.gpsimd.dma_start(cN_dram[layer], c_new[:])
        c_prev = c_new


def build_program(S_=S, debug=False):
    nc = bacc.Bacc(
        "TRN2",
        target_bir_lowering=False,
        debug=debug,
        num_devices=NCORES,
    )
    MT = S_ * BL  # GEMM output rows

    # --- I/O -------------------------------------------------------------
    xT = nc.dram_tensor("xT", [D, MT], F32, kind="ExternalInput")
    wihT = [
        nc.dram_tensor(f"wih{l}T", [D, G4], F32, kind="ExternalInput")
        for l in range(L)
    ]
    whhT = [
        nc.dram_tensor(f"whh{l}T", [H, G4], F32, kind="ExternalInput")
        for l in range(L)
    ]
    bias = [
        nc.dram_tensor(f"bias{l}", [1, G4], F32, kind="ExternalInput")
        for l in range(L)
    ]
    ht0 = [
        nc.dram_tensor(f"ht0_{l}", [128, 4 * BL], F32, kind="ExternalInput")
        for l in range(L)
    ]
    c0 = [
        nc.dram_tensor(f"c0_{l}", [BL, H], F32, kind="ExternalInput")
        for l in range(L)
    ]
    ident_in = nc.dram_tensor("ident8", [BL, BL], F32, kind="ExternalInput")

    out1 = nc.dram_tensor("out1", [MT, H], F32, kind="ExternalOutput")
    hN = nc.dram_tensor("hN", [L, BL, H], F32, kind="ExternalOutput")
    cN = nc.dram_tensor("cN", [L, BL, H], F32, kind="ExternalOutput")

    G0_dram = nc.dram_tensor("G0_i", [MT, G4], F32)
    G1_dram = nc.dram_tensor("G1_i", [MT, G4], F32)
    out0T_dram = nc.dram_tensor("out0T_i", [MT * D], F32)

    with tile.TileContext(nc) as tc:
        with (
            tc.tile_pool(name="const", bufs=1) as constp,
            tc.tile_pool(name="psum", bufs=1, space="PSUM") as psump,
            tc.tile_pool(name="tps", bufs=2, space="PSUM") as tpsp,
            tc.tile_pool(name="w", bufs=1) as wp,
            tc.tile_pool(name="gq", bufs=2) as gqp,
            tc.tile_pool(name="gsb", bufs=2) as gsbp,
            tc.tile_pool(name="lhsT_m", bufs=3) as lhsmp,
            tc.tile_pool(name="blk", bufs=3) as blkp,
            tc.tile_pool(name="act", bufs=2) as actp,
            tc.tile_pool(name="dve", bufs=2) as dvep,
            tc.tile_pool(name="c", bufs=3) as cp,
            tc.tile_pool(name="h", bufs=3) as hp,
            tc.tile_pool(name="ht", bufs=3) as htp,
        ):
            pools = {
                "psum": psump, "tps": tpsp, "gq": gqp, "gsb": gsbp,
                "blk": blkp, "act": actp, "dve": dvep, "c": cp, "h": hp,
                "ht": htp,
            }
            ident8 = constp.tile([BL, BL], F32, tag="ident")
            nc.sync.dma_start(ident8[:], ident_in[:])
            ones1 = constp.tile([1, MT], F32, tag="ones1")
            nc.vector.memset(ones1[:], 1.0)
            bias_sb = [constp.tile([1, G4], F32, tag=f"bias{l}", name=f"bias_sb{l}") for l in range(L)]
            ht_init = [constp.tile([128, 4 * BL], F32, tag=f"ht0_{l}", name=f"ht_init{l}") for l in range(L)]
            c_init = [cp.tile([BL, H], F32, tag="c", name="c_init0")]
            for l in range(L):
                nc.sync.dma_start(bias_sb[l][:], bias[l][:])
                nc.sync.dma_start(ht_init[l][:], ht0[l][:])
            nc.sync.dma_start(c_init[0][:], c0[0][:])

            def load_w(dram, kdim):
                t = wp.tile([128, kdim // 128, G4], F32, tag="wslot")
                nc.sync.dma_start(
                    t[:], dram[:].rearrange("(k p) n -> p k n", p=128)
                )
                return t

            # ---- phase 0: G0 = x @ Wih0^T + b0 --------------------------
            wih0_sb = load_w(wihT[0], D)

            def lhsT_x(m):
                lt = lhsmp.tile([128, 4, 128], F32, tag="lhsm")
                nc.sync.dma_start(
                    lt[:],
                    xT[:, m * 128 : (m + 1) * 128].rearrange(
                        "(k p) m -> p k m", p=128
                    ),
                )
                return [lt[:, k, :] for k in range(4)]

            _emit_gemm(nc, pools, S_, lhsT_x, wih0_sb, bias_sb[0], ones1, G0_dram)

            # ---- phase 1: layer-0 recurrence ----------------------------
            whh0_sb = load_w(whhT[0], H)
            _emit_recurrence(
                nc, pools, S_, 0, whh0_sb, G0_dram, ht_init[0], c_init[0],
                ident8, out0T_dram, None, hN, cN,
            )

            # ---- phase 2: G1 = out0 @ Wih1^T + b1 -----------------------
            wih1_sb = load_w(wihT[1], D)

            def lhsT_o(m):
                lt = lhsmp.tile([128, 4, 128], F32, tag="lhsm")
                nc.sync.dma_start(
                    lt[:],
                    out0T_dram[m * 128 * 512 : (m + 1) * 128 * 512].rearrange(
                        "(p k m) -> p k m", p=128, k=4
                    ),
                )
                return [lt[:, k, :] for k in range(4)]

            _emit_gemm(nc, pools, S_, lhsT_o, wih1_sb, bias_sb[1], ones1, G1_dram)

            # ---- phase 3: layer-1 recurrence ----------------------------
            whh1_sb = load_w(whhT[1], H)
            c_init1 = cp.tile([BL, H], F32, tag="c")
            nc.sync.dma_start(c_init1[:], c0[1][:])
            _emit_recurrence(
                nc, pools, S_, 1, whh1_sb, G1_dram, ht_init[1], c_init1,
                ident8, None, out1, hN, cN,
            )

    nc.compile()
    return nc


# ---------------------------------------------------------------------------
# host side
# ---------------------------------------------------------------------------

def _core_inputs(x, enc_h, enc_c, Wih, Whh, bih, bhh, d, q, S_):
    """Build the in_map for core (direction d in {0 fwd, 1 bwd}, quarter q)."""
    bsl = slice(q * BL, (q + 1) * BL)
    off = 0 if d == 0 else H
    xs = x[bsl, :S_]
    if d == 1:
        xs = xs[:, ::-1]
    m = {}
    m["xT"] = np.ascontiguousarray(
        xs.transpose(2, 1, 0).reshape(D, S_ * BL), dtype=np.float32
    )
    for l in range(L):
        wp_ih = Wih[l][GATE_PERM]
        wp_hh = Whh[l][GATE_PERM]
        bp = (bih[l] + bhh[l])[GATE_PERM]
        m[f"wih{l}T"] = np.ascontiguousarray(wp_ih.T, dtype=np.float32)
        m[f"whh{l}T"] = np.ascontiguousarray(wp_hh.T, dtype=np.float32)
        m[f"bias{l}"] = np.ascontiguousarray(bp[None, :], dtype=np.float32)
        hvec = enc_h[l, bsl, off : off + H]  # [BL, H]
        m[f"ht0_{l}"] = np.ascontiguousarray(
            hvec.T.reshape(4, 128, BL).transpose(1, 0, 2).reshape(128, 4 * BL),
            dtype=np.float32,
        )
        m[f"c0_{l}"] = np.ascontiguousarray(
            enc_c[l, bsl, off : off + H], dtype=np.float32
        )
    m["ident8"] = np.eye(BL, dtype=np.float32)
    return m


_CACHE = {}
LAST_EXEC_NS = None


def _get_program(S_):
    if S_ not in _CACHE:
        _CACHE[S_] = build_program(S_)
    return _CACHE[S_]


def kernel(x, enc_h, enc_c, Wih_f, Whh_f, bih_f, bhh_f, Wih_b, Whh_b, bih_b, bhh_b):
    from concourse.bass_utils import run_bass_kernel_spmd

    x = np.asarray(x, dtype=np.float32)
    enc_h = np.asarray(enc_h, dtype=np.float32)
    enc_c = np.asarray(enc_c, dtype=np.float32)
    Ws = {
        0: (np.asarray(Wih_f, np.float32), np.asarray(Whh_f, np.float32),
            np.asarray(bih_f, np.float32), np.asarray(bhh_f, np.float32)),
        1: (np.asarray(Wih_b, np.float32), np.asarray(Whh_b, np.float32),
            np.asarray(bih_b, np.float32), np.asarray(bhh_b, np.float32)),
    }

    nc = _get_program(S)
    in_maps = []
    for cid in range(NCORES):
        d, q = cid // 4, cid % 4
        wih, whh, bi, bh = Ws[d]
        in_maps.append(
            _core_inputs(x, enc_h, enc_c, wih, whh, bi, bh, d, q, S)
        )
    import os

    want_trace = os.environ.get("KERNEL_TRACE", "0") == "1"
    res_obj = run_bass_kernel_spmd(
        nc, in_maps, list(range(NCORES)), trace=want_trace
    )
    res = res_obj.results
    global LAST_EXEC_NS, LAST_RESULTS
    LAST_RESULTS = res_obj
    if res_obj.exec_time_ns is not None:
        LAST_EXEC_NS = res_obj.exec_time_ns

    out = _assemble(res)
    return out


def _assemble(res):
    out = np.empty((B, S, 2 * H), dtype=np.float32)
    h = np.empty((L, B, 2 * H), dtype=np.float32)
    c = np.empty((L, B, 2 * H), dtype=np.float32)
    for cid in range(NCORES):
        d, q = cid // 4, cid % 4
        bsl = slice(q * BL, (q + 1) * BL)
        off = 0 if d == 0 else H
        r = res[cid]
        o1 = r["out1"].reshape(S, BL, H).transpose(1, 0, 2)  # [BL, S, H]
        if d == 1:
            o1 = o1[:, ::-1]
        out[bsl, :, off : off + H] = o1
        h[:, bsl, off : off + H] = r["hN"]
        c[:, bsl, off : off + H] = r["cN"]
    return out, h, c


def bench(inputs, iters=5):
    """Time pure NEFF executions (compile + transfers excluded).

    Mirrors bass2jax.run_bass_via_pjrt's multi-core path with inputs
    pre-placed on device; returns (best_seconds, per_iter_list, results).
    """
    import time

    import jax
    import jax.numpy as jnp
    from jax.experimental.shard_map import shard_map
    from jax.sharding import Mesh, NamedSharding, PartitionSpec

    from concourse import bass2jax, mybir as mb

    nc = _get_program(S)
    x = np.asarray(inputs["x"], np.float32)
    enc_h = np.asarray(inputs["enc_h"], np.float32)
    enc_c = np.asarray(inputs["enc_c"], np.float32)
    Ws = {
        0: tuple(np.asarray(inputs[k], np.float32)
                 for k in ("Wih_f", "Whh_f", "bih_f", "bhh_f")),
        1: tuple(np.asarray(inputs[k], np.float32)
                 for k in ("Wih_b", "Whh_b", "bih_b", "bhh_b")),
    }
    in_maps = []
    for cid in range(NCORES):
        d, q = cid // 4, cid % 4
        wih, whh, bi, bh = Ws[d]
        in_maps.append(_core_inputs(x, enc_h, enc_c, wih, whh, bi, bh, d, q, S))

    bass2jax.install_neuronx_cc_hook()
    partition_name = (
        nc.partition_id_tensor.name if nc.partition_id_tensor else None
    )
    in_names, out_names, out_avals, zero_outs = [], [], [], []
    for alloc in nc.m.functions[0].allocations:
        if not isinstance(alloc, mb.MemoryLocationSet):
            continue
        name = alloc.memorylocations[0].name
        if alloc.kind == "ExternalInput":
            if name != partition_name:
                in_names.append(name)
        elif alloc.kind == "ExternalOutput":
            out_names.append(name)
            shape = tuple(alloc.tensor_shape)
            dtype = mb.dt.np(alloc.dtype)
            out_avals.append(jax.core.ShapedArray(shape, dtype))
            zero_outs.append(np.zeros(shape, dtype))
    n_params = len(in_names)
    n_outs = len(out_avals)
    all_in_names = list(in_names) + out_names
    if partition_name is not None:
        all_in_names.append(partition_name)
    donate = tuple(range(n_params, n_params + n_outs))

    def _body(*args):
        operands = list(args)
        if partition_name is not None:
            operands.append(bass2jax.partition_id_tensor())
        outs = bass2jax._bass_exec_p.bind(
            *operands,
            out_avals=tuple(out_avals),
            in_names=tuple(all_in_names),
            out_names=tuple(out_names),
            lowering_input_output_aliases=(),
            sim_require_finite=True,
            sim_require_nnan=True,
            nc=nc,
        )
        return tuple(outs)

    devices = jax.devices()[:NCORES]
    mesh = Mesh(np.asarray(devices), ("core",))
    in_specs = (PartitionSpec("core"),) * (n_params + n_outs)
    out_specs = (PartitionSpec("core"),) * n_outs
    sharded = jax.jit(
        shard_map(_body, mesh=mesh, in_specs=in_specs,
                  out_specs=out_specs, check_rep=False),
        donate_argnums=donate, keep_unused=True,
    )
    shard = NamedSharding(mesh, PartitionSpec("core"))
    concat_in = [
        jax.device_put(
            np.concatenate([in_maps[c][n] for c in range(NCORES)], axis=0),
            shard,
        )
        for n in in_names
    ]
    for a in concat_in:
        a.block_until_ready()

    def fresh_zeros():
        return [
            jax.device_put(
                np.zeros((NCORES * z.shape[0], *z.shape[1:]), z.dtype), shard
            )
            for z in zero_outs
        ]

    # warmup (compiles)
    zs = fresh_zeros()
    [a.block_until_ready() for a in zs]
    out_arrs = sharded(*concat_in, *zs)
    jax.block_until_ready(out_arrs)

    times = []
    for _ in range(iters):
        zs = fresh_zeros()
        [a.block_until_ready() for a in zs]
        t0 = time.perf_counter()
        out_arrs = sharded(*concat_in, *zs)
        jax.block_until_ready(out_arrs)
        times.append(time.perf_counter() - t0)

    res = [
        {
            name: np.asarray(out_arrs[i]).reshape(NCORES, *out_avals[i].shape)[c]
            for i, name in enumerate(out_names)
        }
        for c in range(NCORES)
    ]
    return min(times), times, _assemble(res)


# revision 12
# speedup vs baseline: 1.1611x; 1.1611x over previous
# Bidirectional 2-layer LSTM decoder on 8 Trainium2 NeuronCores.
#
# Decomposition: the network factors into independent (batch, direction)
# chains — directions only concatenate at the output, and layer 1 of a
# direction consumes only that direction's layer-0 output. So the 8 cores
# run one uniform SPMD program: core = (direction, batch-quarter), with
# the direction realized purely through per-core data (time-reversed x and
# that direction's weights).
#
# Per core (B_local=8, S=512, H=512):
#   GEMM0:  G0 = x @ Wih0^T + bias     (big matmul, written to DRAM)
#   REC0:   512-step LSTM recurrence, layer 0
#   GEMM1:  G1 = out0 @ Wih1^T + bias
#   REC1:   512-step recurrence, layer 1 -> out, final h/c
#
# Recurrence step (batch-major, gates column order [g|i|f|o]):
#   gates_psum  = I8.T @ G[t]          (identity matmul folds the
#                                       precomputed input term into PSUM)
#   gates_psum += h_{t-1} @ Whh^T      (h^T is the tiny stationary operand;
#                                       the weight matrix streams, which is
#                                       what the PE does at full rate)
#   ACT: tanh(g), sigmoid(i,f), sigmoid(o), tanh(c')
#   DVE: c' = sf*c + si*tg ; h' = so*tanh(c')
#   PE:  4x transpose h' -> h'^T       (stationary operand for step t+1)

import sys

import numpy as np

for _p in ("/opt/trn_rl_repo", "/root/.axon_site/_ro/trn_rl_repo"):
    if _p not in sys.path:
        sys.path.append(_p)

import concourse.bass as bass  # noqa: E402
import concourse.mybir as mybir  # noqa: E402
import concourse.tile as tile  # noqa: E402
from concourse import bacc  # noqa: E402

F32 = mybir.dt.float32
BF16 = mybir.dt.bfloat16
AF = mybir.ActivationFunctionType

H = 512
L = 2
B = 32
S = 512
D = 512
NCORES = 8
BL = B // (NCORES // 2)  # 8: batch rows per core (2 dirs x 4 quarters)
G4 = 4 * H  # 2048 gate columns

# reorder torch gate rows (i,f,g,o) -> (g,i,f,o) so tanh(g) input is ready
# first in the matmul stream and sigmoid(i,f) reads one contiguous slab
GATE_PERM = np.r_[2 * H : 3 * H, 0:H, H : 2 * H, 3 * H : 4 * H]
SL_G = slice(0, H)
SL_IF = slice(H, 3 * H)
SL_I = slice(H, 2 * H)
SL_F = slice(2 * H, 3 * H)
SL_O = slice(3 * H, 4 * H)


def _emit_gemm(nc, pools, S_, lhsT_src, rhs_sb, bias_sb, ones1, G_dram):
    """G_dram[m*128:(m+1)*128, :] = lhsT_m.T @ rhs (+ ones1.T @ bias row).

    lhsT_src(m) -> list of 4 [128,128] APs (K-chunks of the stationary
    operand for output row-tile m). rhs_sb is [128, 4, G4] in SBUF.
    """
    n_m = (S_ * BL) // 128
    for m in range(n_m):
        lhsT = lhsT_src(m)
        gout = pools["gsb"].tile([128, G4], BF16, tag="gsb")
        for n in range(4):
            ps = pools["psum_g"].tile([128, H], F32, tag="ps_gem", name="psg")
            nc.tensor.matmul(
                ps[:],
                ones1[:, m * 128 : (m + 1) * 128],
                bias_sb[:, n * H : (n + 1) * H],
                start=True,
                stop=False,
            )
            for k in range(4):
                nc.tensor.matmul(
                    ps[:],
                    lhsT[k],
                    rhs_sb[:, k, n * H : (n + 1) * H],
                    start=False,
                    stop=(k == 3),
                )
            dst = gout[:, n * H : (n + 1) * H]
            if n % 2 == 0:
                nc.scalar.copy(dst, ps[:])
            else:
                nc.vector.tensor_copy(dst, ps[:])
        nc.gpsimd.dma_start(G_dram[m * 128 : (m + 1) * 128, :], gout[:])


def _emit_recurrence(
    nc, pools, S_, layer, whh_sb, G_dram, ht_init, c_init, ident8, ident8b,
    out0T_dram, out1_dram, hN_dram, cN_dram,
):
    """One 512-step LSTM chain. layer 0 stores h^T blocks (GEMM1 stationary);
    layer 1 stores the output sequence and both layers store final h/c."""
    blk = None
    ht_prev = None  # layer 1: rotating [128, 32] h^T tile
    prev_blk, prev_off = None, 0  # layer 0: block tile holding h^T_{t-1}
    c_prev = c_init
    for t in range(S_):
        g_sb = pools["gq"].tile([BL, G4], BF16, tag="gq")
        for n in range(4):
            nc.sync.dma_start(
                g_sb[:, n * H : (n + 1) * H],
                G_dram[t * BL : (t + 1) * BL, n * H : (n + 1) * H],
            )

        gates = pools["psum"].tile([BL, G4], F32, tag="ps_main")
        for n in range(4):
            nc.tensor.matmul(
                gates[:, n * H : (n + 1) * H],
                ident8b[:],
                g_sb[:, n * H : (n + 1) * H],
                start=True,
                stop=False,
            )
        for k in range(4):
            if t == 0:
                lhsT = ht_init[:, k * BL : (k + 1) * BL]
            elif layer == 0:
                lhsT = prev_blk[:, k, prev_off * BL : (prev_off + 1) * BL]
            else:
                lhsT = ht_prev[:, k * BL : (k + 1) * BL]
            for n in range(4):
                nc.tensor.matmul(
                    gates[:, n * H : (n + 1) * H],
                    lhsT,
                    whh_sb[:, k, n * H : (n + 1) * H],
                    start=False,
                    stop=(k == 3),
                )

        tg = pools["act"].tile([BL, H], F32, tag="tg")
        nc.scalar.activation(tg[:], gates[:, SL_G], AF.Tanh)
        sifo = pools["act"].tile([BL, 3 * H], F32, tag="sifo")
        nc.scalar.activation(sifo[:], gates[:, H:G4], AF.Sigmoid)
        sif = sifo
        so = sifo[:, 2 * H : 3 * H]

        tmp1 = pools["dve"].tile([BL, H], F32, tag="tmp1")
        nc.vector.tensor_mul(tmp1[:], sif[:, H : 2 * H], c_prev[:])
        tmp2 = pools["dve"].tile([BL, H], F32, tag="tmp2")
        nc.vector.tensor_mul(tmp2[:], sif[:, 0:H], tg[:])
        c_new = pools["c"].tile([BL, H], F32, tag="c")
        nc.vector.tensor_add(c_new[:], tmp1[:], tmp2[:])
        tc_t = pools["act"].tile([BL, H], F32, tag="tc")
        nc.scalar.activation(tc_t[:], c_new[:], AF.Tanh)
        h_new = pools["h"].tile([BL, H], F32, tag="h")
        nc.vector.tensor_mul(h_new[:], so, tc_t[:])

        # h' -> h'^T (4 x [8,128] -> [128,8] PE transposes, one ACT copy out)
        tps = pools["tps"].tile([128, 4 * BL], F32, tag="tps")
        for k in range(4):
            nc.tensor.transpose(
                tps[:, k * BL : (k + 1) * BL],
                h_new[:, k * 128 : (k + 1) * 128],
                ident8[:],
            )
        if layer == 0:
            off = t % 16
            if off == 0:
                blk = pools["blk"].tile([128, 4, 16 * BL], BF16, tag="blk")
            dst = blk[:, :, off * BL : (off + 1) * BL]
            nc.scalar.copy(dst, tps[:].rearrange("p (k b) -> p k b", b=BL))
            if off == 15:
                m = t // 16
                nblk = 128 * 4 * 16 * BL  # elems per 16-step h^T block
                nc.gpsimd.dma_start(
                    out0T_dram[m * nblk : (m + 1) * nblk]
                    .rearrange("(p k b) -> p k b", p=128, k=4),
                    blk[:],
                )
            prev_blk, prev_off = blk, off
        else:
            ht_new = pools["ht"].tile([128, 4 * BL], BF16, tag="ht")
            nc.scalar.copy(ht_new[:], tps[:])
            ht_prev = ht_new
            nc.gpsimd.dma_start(out1_dram[t * BL : (t + 1) * BL, :], h_new[:])

        if t == S_ - 1:
            nc.gpsimd.dma_start(hN_dram[layer], h_new[:])
            nc.gpsimd.dma_start(cN_dram[layer], c_new[:])
        c_prev = c_new


def build_program(S_=S, debug=False):
    nc = bacc.Bacc(
        "TRN2",
        target_bir_lowering=False,
        debug=debug,
        num_devices=NCORES,
    )
    MT = S_ * BL  # GEMM output rows

    # --- I/O -------------------------------------------------------------
    xT = nc.dram_tensor("xT", [D, MT], BF16, kind="ExternalInput")
    wihT = [
        nc.dram_tensor(f"wih{l}T", [D, G4], BF16, kind="ExternalInput")
        for l in range(L)
    ]
    whhT = [
        nc.dram_tensor(f"whh{l}T", [H, G4], BF16, kind="ExternalInput")
        for l in range(L)
    ]
    bias = [
        nc.dram_tensor(f"bias{l}", [1, G4], BF16, kind="ExternalInput")
        for l in range(L)
    ]
    ht0 = [
        nc.dram_tensor(f"ht0_{l}", [128, 4 * BL], BF16, kind="ExternalInput")
        for l in range(L)
    ]
    c0 = [
        nc.dram_tensor(f"c0_{l}", [BL, H], F32, kind="ExternalInput")
        for l in range(L)
    ]
    ident_in = nc.dram_tensor("ident8", [BL, BL], F32, kind="ExternalInput")
    identb_in = nc.dram_tensor("ident8b", [BL, BL], BF16, kind="ExternalInput")

    out1 = nc.dram_tensor("out1", [MT, H], F32, kind="ExternalOutput")
    hN = nc.dram_tensor("hN", [L, BL, H], F32, kind="ExternalOutput")
    cN = nc.dram_tensor("cN", [L, BL, H], F32, kind="ExternalOutput")

    G0_dram = nc.dram_tensor("G0_i", [MT, G4], BF16)
    G1_dram = nc.dram_tensor("G1_i", [MT, G4], BF16)
    out0T_dram = nc.dram_tensor("out0T_i", [MT * D], BF16)

    with tile.TileContext(nc) as tc:
        with (
            tc.tile_pool(name="const", bufs=1) as constp,
            tc.tile_pool(name="psum", bufs=1, space="PSUM") as psump,
            tc.tile_pool(name="psum_g", bufs=2, space="PSUM") as psumgp,
            tc.tile_pool(name="tps", bufs=2, space="PSUM") as tpsp,
            tc.tile_pool(name="w", bufs=1) as wp,
            tc.tile_pool(name="gq", bufs=3) as gqp,
            tc.tile_pool(name="gsb", bufs=2) as gsbp,
            tc.tile_pool(name="lhsT_m", bufs=3) as lhsmp,
            tc.tile_pool(name="blk", bufs=3) as blkp,
            tc.tile_pool(name="act", bufs=2) as actp,
            tc.tile_pool(name="dve", bufs=2) as dvep,
            tc.tile_pool(name="c", bufs=3) as cp,
            tc.tile_pool(name="h", bufs=3) as hp,
            tc.tile_pool(name="ht", bufs=3) as htp,
        ):
            pools = {
                "psum": psump, "psum_g": psumgp, "tps": tpsp, "gq": gqp,
                "gsb": gsbp,
                "blk": blkp, "act": actp, "dve": dvep, "c": cp, "h": hp,
                "ht": htp,
            }
            ident8 = constp.tile([BL, BL], F32, tag="ident")
            nc.sync.dma_start(ident8[:], ident_in[:])
            ident8b = constp.tile([BL, BL], BF16, tag="identb")
            nc.sync.dma_start(ident8b[:], identb_in[:])
            ones1 = constp.tile([1, MT], BF16, tag="ones1")
            nc.vector.memset(ones1[:], 1.0)
            bias_sb = [constp.tile([1, G4], BF16, tag=f"bias{l}", name=f"bias_sb{l}") for l in range(L)]
            ht_init = [constp.tile([128, 4 * BL], BF16, tag=f"ht0_{l}", name=f"ht_init{l}") for l in range(L)]
            c_init = [cp.tile([BL, H], F32, tag="c", name="c_init0")]
            for l in range(L):
                nc.sync.dma_start(bias_sb[l][:], bias[l][:])
                nc.sync.dma_start(ht_init[l][:], ht0[l][:])
            nc.sync.dma_start(c_init[0][:], c0[0][:])

            def load_w(dram, kdim):
                t = wp.tile([128, kdim // 128, G4], BF16, tag="wslot")
                nc.sync.dma_start(
                    t[:], dram[:].rearrange("(k p) n -> p k n", p=128)
                )
                return t

            # ---- phase 0: G0 = x @ Wih0^T + b0 --------------------------
            wih0_sb = load_w(wihT[0], D)

            def lhsT_x(m):
                lt = lhsmp.tile([128, 4, 128], BF16, tag="lhsm")
                nc.sync.dma_start(
                    lt[:],
                    xT[:, m * 128 : (m + 1) * 128].rearrange(
                        "(k p) m -> p k m", p=128
                    ),
                )
                return [lt[:, k, :] for k in range(4)]

            _emit_gemm(nc, pools, S_, lhsT_x, wih0_sb, bias_sb[0], ones1, G0_dram)

            # ---- phase 1: layer-0 recurrence ----------------------------
            whh0_sb = load_w(whhT[0], H)
            _emit_recurrence(
                nc, pools, S_, 0, whh0_sb, G0_dram, ht_init[0], c_init[0],
                ident8, ident8b, out0T_dram, None, hN, cN,
            )

            # ---- phase 2: G1 = out0 @ Wih1^T + b1 -----------------------
            wih1_sb = load_w(wihT[1], D)

            def lhsT_o(m):
                lt = lhsmp.tile([128, 4, 128], BF16, tag="lhsm")
                nc.sync.dma_start(
                    lt[:],
                    out0T_dram[m * 128 * 512 : (m + 1) * 128 * 512].rearrange(
                        "(p k m) -> p k m", p=128, k=4
                    ),
                )
                return [lt[:, k, :] for k in range(4)]

            _emit_gemm(nc, pools, S_, lhsT_o, wih1_sb, bias_sb[1], ones1, G1_dram)

            # ---- phase 3: layer-1 recurrence ----------------------------
            whh1_sb = load_w(whhT[1], H)
            c_init1 = cp.tile([BL, H], F32, tag="c")
            nc.sync.dma_start(c_init1[:], c0[1][:])
            _emit_recurrence(
                nc, pools, S_, 1, whh1_sb, G1_dram, ht_init[1], c_init1,
                ident8, ident8b, None, out1, hN, cN,
            )

    nc.compile()
    return nc


# ---------------------------------------------------------------------------
# host side
# ---------------------------------------------------------------------------

def _core_inputs(x, enc_h, enc_c, Wih, Whh, bih, bhh, d, q, S_):
    """Build the in_map for core (direction d in {0 fwd, 1 bwd}, quarter q)."""
    bsl = slice(q * BL, (q + 1) * BL)
    off = 0 if d == 0 else H
    xs = x[bsl, :S_]
    if d == 1:
        xs = xs[:, ::-1]
    m = {}
    import ml_dtypes

    bf = ml_dtypes.bfloat16
    m["xT"] = np.ascontiguousarray(
        xs.transpose(2, 1, 0).reshape(D, S_ * BL)
    ).astype(bf)
    for l in range(L):
        wp_ih = Wih[l][GATE_PERM]
        wp_hh = Whh[l][GATE_PERM]
        bp = (bih[l] + bhh[l])[GATE_PERM]
        m[f"wih{l}T"] = np.ascontiguousarray(wp_ih.T).astype(bf)
        m[f"whh{l}T"] = np.ascontiguousarray(wp_hh.T).astype(bf)
        m[f"bias{l}"] = np.ascontiguousarray(bp[None, :]).astype(bf)
        hvec = enc_h[l, bsl, off : off + H]  # [BL, H]
        m[f"ht0_{l}"] = np.ascontiguousarray(
            hvec.T.reshape(4, 128, BL).transpose(1, 0, 2).reshape(128, 4 * BL)
        ).astype(bf)
        m[f"c0_{l}"] = np.ascontiguousarray(
            enc_c[l, bsl, off : off + H], dtype=np.float32
        )
    m["ident8"] = np.eye(BL, dtype=np.float32)
    m["ident8b"] = np.eye(BL).astype(bf)
    return m


_CACHE = {}
LAST_EXEC_NS = None


def _get_program(S_):
    if S_ not in _CACHE:
        _CACHE[S_] = build_program(S_)
    return _CACHE[S_]


def kernel(x, enc_h, enc_c, Wih_f, Whh_f, bih_f, bhh_f, Wih_b, Whh_b, bih_b, bhh_b):
    from concourse.bass_utils import run_bass_kernel_spmd

    x = np.asarray(x, dtype=np.float32)
    enc_h = np.asarray(enc_h, dtype=np.float32)
    enc_c = np.asarray(enc_c, dtype=np.float32)
    Ws = {
        0: (np.asarray(Wih_f, np.float32), np.asarray(Whh_f, np.float32),
            np.asarray(bih_f, np.float32), np.asarray(bhh_f, np.float32)),
        1: (np.asarray(Wih_b, np.float32), np.asarray(Whh_b, np.float32),
            np.asarray(bih_b, np.float32), np.asarray(bhh_b, np.float32)),
    }

    nc = _get_program(S)
    in_maps = []
    for cid in range(NCORES):
        d, q = cid // 4, cid % 4
        wih, whh, bi, bh = Ws[d]
        in_maps.append(
            _core_inputs(x, enc_h, enc_c, wih, whh, bi, bh, d, q, S)
        )
    import os

    want_trace = os.environ.get("KERNEL_TRACE", "0") == "1"
    res_obj = run_bass_kernel_spmd(
        nc, in_maps, list(range(NCORES)), trace=want_trace
    )
    res = res_obj.results
    global LAST_EXEC_NS, LAST_RESULTS
    LAST_RESULTS = res_obj
    if res_obj.exec_time_ns is not None:
        LAST_EXEC_NS = res_obj.exec_time_ns

    out = _assemble(res)
    return out


def _assemble(res):
    out = np.empty((B, S, 2 * H), dtype=np.float32)
    h = np.empty((L, B, 2 * H), dtype=np.float32)
    c = np.empty((L, B, 2 * H), dtype=np.float32)
    for cid in range(NCORES):
        d, q = cid // 4, cid % 4
        bsl = slice(q * BL, (q + 1) * BL)
        off = 0 if d == 0 else H
        r = res[cid]
        o1 = r["out1"].reshape(S, BL, H).transpose(1, 0, 2)  # [BL, S, H]
        if d == 1:
            o1 = o1[:, ::-1]
        out[bsl, :, off : off + H] = o1
        h[:, bsl, off : off + H] = r["hN"]
        c[:, bsl, off : off + H] = r["cN"]
    return out, h, c


def bench(inputs, iters=5, S_=S):
    """Time pure NEFF executions (compile + transfers excluded).

    Mirrors bass2jax.run_bass_via_pjrt's multi-core path with inputs
    pre-placed on device; returns (best_seconds, per_iter_list, results).
    """
    import time

    import jax
    import jax.numpy as jnp
    from jax.experimental.shard_map import shard_map
    from jax.sharding import Mesh, NamedSharding, PartitionSpec

    from concourse import bass2jax, mybir as mb

    nc = _get_program(S_)
    x = np.asarray(inputs["x"], np.float32)
    enc_h = np.asarray(inputs["enc_h"], np.float32)
    enc_c = np.asarray(inputs["enc_c"], np.float32)
    Ws = {
        0: tuple(np.asarray(inputs[k], np.float32)
                 for k in ("Wih_f", "Whh_f", "bih_f", "bhh_f")),
        1: tuple(np.asarray(inputs[k], np.float32)
                 for k in ("Wih_b", "Whh_b", "bih_b", "bhh_b")),
    }
    in_maps = []
    for cid in range(NCORES):
        d, q = cid // 4, cid % 4
        wih, whh, bi, bh = Ws[d]
        in_maps.append(_core_inputs(x, enc_h, enc_c, wih, whh, bi, bh, d, q, S_))

    bass2jax.install_neuronx_cc_hook()
    partition_name = (
        nc.partition_id_tensor.name if nc.partition_id_tensor else None
    )
    in_names, out_names, out_avals, zero_outs = [], [], [], []
    for alloc in nc.m.functions[0].allocations:
        if not isinstance(alloc, mb.MemoryLocationSet):
            continue
        name = alloc.memorylocations[0].name
        if alloc.kind == "ExternalInput":
            if name != partition_name:
                in_names.append(name)
        elif alloc.kind == "ExternalOutput":
            out_names.append(name)
            shape = tuple(alloc.tensor_shape)
            dtype = mb.dt.np(alloc.dtype)
            out_avals.append(jax.core.ShapedArray(shape, dtype))
            zero_outs.append(np.zeros(shape, dtype))
    n_params = len(in_names)
    n_outs = len(out_avals)
    all_in_names = list(in_names) + out_names
    if partition_name is not None:
        all_in_names.append(partition_name)
    donate = tuple(range(n_params, n_params + n_outs))

    def _body(*args):
        operands = list(args)
        if partition_name is not None:
            operands.append(bass2jax.partition_id_tensor())
        outs = bass2jax._bass_exec_p.bind(
            *operands,
            out_avals=tuple(out_avals),
            in_names=tuple(all_in_names),
            out_names=tuple(out_names),
            lowering_input_output_aliases=(),
            sim_require_finite=True,
            sim_require_nnan=True,
            nc=nc,
        )
        return tuple(outs)

    devices = jax.devices()[:NCORES]
    mesh = Mesh(np.asarray(devices), ("core",))
    in_specs = (PartitionSpec("core"),) * (n_params + n_outs)
    out_specs = (PartitionSpec("core"),) * n_outs
    sharded = jax.jit(
        shard_map(_body, mesh=mesh, in_specs=in_specs,
                  out_specs=out_specs, check_rep=False),
        donate_argnums=donate, keep_unused=True,
    )
    shard = NamedSharding(mesh, PartitionSpec("core"))
    concat_in = [
        jax.device_put(
            np.concatenate([in_maps[c][n] for c in range(NCORES)], axis=0),
            shard,
        )
        for n in in_names
    ]
    for a in concat_in:
        a.block_until_ready()

    def fresh_zeros():
        return [
            jax.device_put(
                np.zeros((NCORES * z.shape[0], *z.shape[1:]), z.dtype), shard
            )
            for z in zero_outs
        ]

    # warmup (compiles)
    zs = fresh_zeros()
    [a.block_until_ready() for a in zs]
    out_arrs = sharded(*concat_in, *zs)
    jax.block_until_ready(out_arrs)

    times = []
    for _ in range(iters):
        zs = fresh_zeros()
        [a.block_until_ready() for a in zs]
        t0 = time.perf_counter()
        out_arrs = sharded(*concat_in, *zs)
        jax.block_until_ready(out_arrs)
        times.append(time.perf_counter() - t0)

    res = [
        {
            name: np.asarray(out_arrs[i]).reshape(NCORES, *out_avals[i].shape)[c]
            for i, name in enumerate(out_names)
        }
        for c in range(NCORES)
    ]
    return min(times), times, (_assemble(res) if S_ == S else res)


# revision 14
# speedup vs baseline: 1.1948x; 1.0290x over previous
# Bidirectional 2-layer LSTM decoder on 8 Trainium2 NeuronCores.
#
# Decomposition: the network factors into independent (batch, direction)
# chains — directions only concatenate at the output, and layer 1 of a
# direction consumes only that direction's layer-0 output. So the 8 cores
# run one uniform SPMD program: core = (direction, batch-quarter), with
# the direction realized purely through per-core data (time-reversed x and
# that direction's weights).
#
# Per core (B_local=8, S=512, H=512):
#   GEMM0:  G0 = x @ Wih0^T + bias     (big matmul, written to DRAM)
#   REC0:   512-step LSTM recurrence, layer 0
#   GEMM1:  G1 = out0 @ Wih1^T + bias
#   REC1:   512-step recurrence, layer 1 -> out, final h/c
#
# Recurrence step (batch-major, gates column order [g|i|f|o]):
#   gates_psum  = I8.T @ G[t]          (identity matmul folds the
#                                       precomputed input term into PSUM)
#   gates_psum += h_{t-1} @ Whh^T      (h^T is the tiny stationary operand;
#                                       the weight matrix streams, which is
#                                       what the PE does at full rate)
#   ACT: tanh(g), sigmoid(i,f), sigmoid(o), tanh(c')
#   DVE: c' = sf*c + si*tg ; h' = so*tanh(c')
#   PE:  4x transpose h' -> h'^T       (stationary operand for step t+1)

import sys

import numpy as np

for _p in ("/opt/trn_rl_repo", "/root/.axon_site/_ro/trn_rl_repo"):
    if _p not in sys.path:
        sys.path.append(_p)

import concourse.bass as bass  # noqa: E402
import concourse.mybir as mybir  # noqa: E402
import concourse.tile as tile  # noqa: E402
from concourse import bacc  # noqa: E402

F32 = mybir.dt.float32
BF16 = mybir.dt.bfloat16
AF = mybir.ActivationFunctionType

H = 512
L = 2
B = 32
S = 512
D = 512
NCORES = 8
BL = B // (NCORES // 2)  # 8: batch rows per core (2 dirs x 4 quarters)
G4 = 4 * H  # 2048 gate columns

# reorder torch gate rows (i,f,g,o) -> (g,i,f,o) so tanh(g) input is ready
# first in the matmul stream and sigmoid(i,f) reads one contiguous slab
GATE_PERM = np.r_[2 * H : 3 * H, 0:H, H : 2 * H, 3 * H : 4 * H]
SL_G = slice(0, H)
SL_IF = slice(H, 3 * H)
SL_I = slice(H, 2 * H)
SL_F = slice(2 * H, 3 * H)
SL_O = slice(3 * H, 4 * H)


def _emit_gemm(nc, pools, S_, lhsT_src, rhs_sb, bias_sb, ones1, G_dram):
    """G_dram[m*128:(m+1)*128, :] = lhsT_m.T @ rhs (+ ones1.T @ bias row).

    lhsT_src(m) -> list of 4 [128,128] APs (K-chunks of the stationary
    operand for output row-tile m). rhs_sb is [128, 4, G4] in SBUF.
    """
    n_m = (S_ * BL) // 128
    for m in range(n_m):
        lhsT = lhsT_src(m)
        gout = pools["gsb"].tile([128, G4], BF16, tag="gsb")
        for n in range(4):
            ps = pools["psum_g"].tile([128, H], F32, tag="ps_gem", name="psg")
            nc.tensor.matmul(
                ps[:],
                ones1[:, m * 128 : (m + 1) * 128],
                bias_sb[:, n * H : (n + 1) * H],
                start=True,
                stop=False,
            )
            for k in range(4):
                nc.tensor.matmul(
                    ps[:],
                    lhsT[k],
                    rhs_sb[:, k, n * H : (n + 1) * H],
                    start=False,
                    stop=(k == 3),
                )
            dst = gout[:, n * H : (n + 1) * H]
            if n % 2 == 0:
                nc.scalar.copy(dst, ps[:])
            else:
                nc.vector.tensor_copy(dst, ps[:])
        nc.gpsimd.dma_start(G_dram[m * 128 : (m + 1) * 128, :], gout[:])


def _emit_recurrence(
    nc, tc, cid, S_, layer, whh_sb, G_dram, ht_init, c_init_dram, ident8,
    ident8b, out0T_dram, out1_dram, hN_dram, cN_dram, ctx,
):
    """One 512-step LSTM chain with chain-private pools so two chains can
    interleave. Gates are computed in two [8,1024] PSUM halves (bank budget):
    half A = [g|i], half B = [f|o]."""
    HH = 2 * H
    ps = ctx.enter_context(tc.tile_pool(name=f"ps{cid}", bufs=1, space="PSUM"))
    tpsp = ctx.enter_context(tc.tile_pool(name=f"tps{cid}", bufs=1, space="PSUM"))
    gqp = ctx.enter_context(tc.tile_pool(name=f"gq{cid}", bufs=4))
    actp = ctx.enter_context(tc.tile_pool(name=f"act{cid}", bufs=2))
    dvep = ctx.enter_context(tc.tile_pool(name=f"dve{cid}", bufs=2))
    cp = ctx.enter_context(tc.tile_pool(name=f"c{cid}", bufs=3))
    hp = ctx.enter_context(tc.tile_pool(name=f"h{cid}", bufs=3))
    htp = ctx.enter_context(tc.tile_pool(name=f"ht{cid}", bufs=3))
    blkp = ctx.enter_context(tc.tile_pool(name=f"blk{cid}", bufs=3))

    c_init = cp.tile([BL, H], F32, tag="c", name=f"c_init{cid}")
    nc.sync.dma_start(c_init[:], c_init_dram[:])

    blk = None
    ht_prev = None
    prev_blk, prev_off = None, 0
    c_prev = c_init
    for t in range(S_):
        g_sb = gqp.tile([BL, G4], BF16, tag="gq", name=f"g_sb{cid}")
        for n in range(4):
            nc.sync.dma_start(
                g_sb[:, n * H : (n + 1) * H],
                G_dram[t * BL : (t + 1) * BL, n * H : (n + 1) * H],
            )

        def lhsT_k(k):
            if t == 0:
                return ht_init[:, k * BL : (k + 1) * BL]
            if layer == 0:
                return prev_blk[:, k, prev_off * BL : (prev_off + 1) * BL]
            return ht_prev[:, k * BL : (k + 1) * BL]

        halves = []
        for hf in range(2):  # A=[g|i], B=[f|o]
            gh = ps.tile([BL, HH], F32, tag="gates", name=f"gates{cid}")
            for n in range(2):
                nc.tensor.matmul(
                    gh[:, n * H : (n + 1) * H],
                    ident8b[:],
                    g_sb[:, (2 * hf + n) * H : (2 * hf + n + 1) * H],
                    start=True,
                    stop=False,
                )
            for k in range(4):
                lh = lhsT_k(k)
                for n in range(2):
                    nc.tensor.matmul(
                        gh[:, n * H : (n + 1) * H],
                        lh,
                        whh_sb[:, k, (2 * hf + n) * H : (2 * hf + n + 1) * H],
                        start=False,
                        stop=(k == 3),
                    )
            halves.append(gh)
            if hf == 0:
                tg = actp.tile([BL, H], F32, tag="tg", name=f"tg{cid}")
                nc.scalar.activation(tg[:], gh[:, 0:H], AF.Tanh)
                si = actp.tile([BL, H], F32, tag="si", name=f"si{cid}")
                nc.scalar.activation(si[:], gh[:, H:HH], AF.Sigmoid)
            else:
                sfo = actp.tile([BL, HH], F32, tag="sfo", name=f"sfo{cid}")
                nc.scalar.activation(sfo[:], gh[:], AF.Sigmoid)

        tmp1 = dvep.tile([BL, H], F32, tag="tmp1", name=f"tmp1_{cid}")
        nc.vector.tensor_mul(tmp1[:], sfo[:, 0:H], c_prev[:])
        tmp2 = dvep.tile([BL, H], F32, tag="tmp2", name=f"tmp2_{cid}")
        nc.vector.tensor_mul(tmp2[:], si[:], tg[:])
        c_new = cp.tile([BL, H], F32, tag="c", name=f"c{cid}")
        nc.vector.tensor_add(c_new[:], tmp1[:], tmp2[:])
        tc_t = actp.tile([BL, H], F32, tag="tc", name=f"tc{cid}")
        nc.scalar.activation(tc_t[:], c_new[:], AF.Tanh)
        h_new = hp.tile([BL, H], F32, tag="h", name=f"h{cid}")
        nc.vector.tensor_mul(h_new[:], sfo[:, H:HH], tc_t[:])

        tps = tpsp.tile([128, 4 * BL], F32, tag="tps", name=f"tps{cid}")
        for k in range(4):
            nc.tensor.transpose(
                tps[:, k * BL : (k + 1) * BL],
                h_new[:, k * 128 : (k + 1) * 128],
                ident8[:],
            )
        if layer == 0:
            off = t % 16
            if off == 0:
                blk = blkp.tile([128, 4, 16 * BL], BF16, tag="blk", name="blk")
            dst = blk[:, :, off * BL : (off + 1) * BL]
            nc.scalar.copy(dst, tps[:].rearrange("p (k b) -> p k b", b=BL))
            if off == 15:
                m = t // 16
                nblk = 128 * 4 * 16 * BL
                nc.gpsimd.dma_start(
                    out0T_dram[m * nblk : (m + 1) * nblk]
                    .rearrange("(p k b) -> p k b", p=128, k=4),
                    blk[:],
                )
            prev_blk, prev_off = blk, off
        else:
            ht_new = htp.tile([128, 4 * BL], BF16, tag="ht", name=f"ht{cid}")
            nc.scalar.copy(ht_new[:], tps[:])
            ht_prev = ht_new
            nc.gpsimd.dma_start(out1_dram[t * BL : (t + 1) * BL, :], h_new[:])

        if t == S_ - 1:
            nc.gpsimd.dma_start(hN_dram[layer], h_new[:])
            nc.gpsimd.dma_start(cN_dram[layer], c_new[:])
        c_prev = c_new


def build_program(S_=S, debug=False):
    nc = bacc.Bacc(
        "TRN2",
        target_bir_lowering=False,
        debug=debug,
        num_devices=NCORES,
    )
    MT = S_ * BL  # GEMM output rows

    # --- I/O -------------------------------------------------------------
    xT = nc.dram_tensor("xT", [D, MT], BF16, kind="ExternalInput")
    wihT = [
        nc.dram_tensor(f"wih{l}T", [D, G4], BF16, kind="ExternalInput")
        for l in range(L)
    ]
    whhT = [
        nc.dram_tensor(f"whh{l}T", [H, G4], BF16, kind="ExternalInput")
        for l in range(L)
    ]
    bias = [
        nc.dram_tensor(f"bias{l}", [1, G4], BF16, kind="ExternalInput")
        for l in range(L)
    ]
    ht0 = [
        nc.dram_tensor(f"ht0_{l}", [128, 4 * BL], BF16, kind="ExternalInput")
        for l in range(L)
    ]
    c0 = [
        nc.dram_tensor(f"c0_{l}", [BL, H], F32, kind="ExternalInput")
        for l in range(L)
    ]
    ident_in = nc.dram_tensor("ident8", [BL, BL], F32, kind="ExternalInput")
    identb_in = nc.dram_tensor("ident8b", [BL, BL], BF16, kind="ExternalInput")

    out1 = nc.dram_tensor("out1", [MT, H], F32, kind="ExternalOutput")
    hN = nc.dram_tensor("hN", [L, BL, H], F32, kind="ExternalOutput")
    cN = nc.dram_tensor("cN", [L, BL, H], F32, kind="ExternalOutput")

    G0_dram = nc.dram_tensor("G0_i", [MT, G4], BF16)
    G1_dram = nc.dram_tensor("G1_i", [MT, G4], BF16)
    out0T_dram = nc.dram_tensor("out0T_i", [MT * D], BF16)

    with tile.TileContext(nc) as tc:
        from contextlib import ExitStack

        ctx = ExitStack()
        with (
            tc.tile_pool(name="const", bufs=1) as constp,
            tc.tile_pool(name="psum_g", bufs=2, space="PSUM") as psumgp,
            tc.tile_pool(name="w", bufs=1) as wp,
            tc.tile_pool(name="gsb", bufs=2) as gsbp,
            tc.tile_pool(name="lhsT_m", bufs=3) as lhsmp,
            ctx,
        ):
            pools = {"psum_g": psumgp, "gsb": gsbp}
            ident8 = constp.tile([BL, BL], F32, tag="ident")
            nc.sync.dma_start(ident8[:], ident_in[:])
            ident8b = constp.tile([BL, BL], BF16, tag="identb")
            nc.sync.dma_start(ident8b[:], identb_in[:])
            ones1 = constp.tile([1, MT], BF16, tag="ones1")
            nc.vector.memset(ones1[:], 1.0)
            bias_sb = [constp.tile([1, G4], BF16, tag=f"bias{l}", name=f"bias_sb{l}") for l in range(L)]
            ht_init = [constp.tile([128, 4 * BL], BF16, tag=f"ht0_{l}", name=f"ht_init{l}") for l in range(L)]
            for l in range(L):
                nc.sync.dma_start(bias_sb[l][:], bias[l][:])
                nc.sync.dma_start(ht_init[l][:], ht0[l][:])

            def load_w(dram, kdim, tag):
                t = wp.tile([128, kdim // 128, G4], BF16, tag=tag, name=tag)
                nc.sync.dma_start(
                    t[:], dram[:].rearrange("(k p) n -> p k n", p=128)
                )
                return t

            # ---- phase 0: G0 = x @ Wih0^T + b0 --------------------------
            wih0_sb = load_w(wihT[0], D, "w_ih0")

            def lhsT_x(m):
                lt = lhsmp.tile([128, 4, 128], BF16, tag="lhsm")
                nc.sync.dma_start(
                    lt[:],
                    xT[:, m * 128 : (m + 1) * 128].rearrange(
                        "(k p) m -> p k m", p=128
                    ),
                )
                return [lt[:, k, :] for k in range(4)]

            _emit_gemm(nc, pools, S_, lhsT_x, wih0_sb, bias_sb[0], ones1, G0_dram)

            # ---- phase 1: layer-0 recurrence ----------------------------
            whh0_sb = load_w(whhT[0], H, "w_hh0")
            _emit_recurrence(
                nc, tc, 0, S_, 0, whh0_sb, G0_dram, ht_init[0], c0[0],
                ident8, ident8b, out0T_dram, None, hN, cN, ctx,
            )

            # ---- phase 2: G1 = out0 @ Wih1^T + b1 -----------------------
            wih1_sb = load_w(wihT[1], D, "w_ih1")

            def lhsT_o(m):
                lt = lhsmp.tile([128, 4, 128], BF16, tag="lhsm")
                nc.sync.dma_start(
                    lt[:],
                    out0T_dram[m * 128 * 512 : (m + 1) * 128 * 512].rearrange(
                        "(p k m) -> p k m", p=128, k=4
                    ),
                )
                return [lt[:, k, :] for k in range(4)]

            _emit_gemm(nc, pools, S_, lhsT_o, wih1_sb, bias_sb[1], ones1, G1_dram)

            # ---- phase 3: layer-1 recurrence ----------------------------
            whh1_sb = load_w(whhT[1], H, "w_hh1")
            _emit_recurrence(
                nc, tc, 1, S_, 1, whh1_sb, G1_dram, ht_init[1], c0[1],
                ident8, ident8b, None, out1, hN, cN, ctx,
            )

    nc.compile()
    return nc


# ---------------------------------------------------------------------------
# host side
# ---------------------------------------------------------------------------

def _core_inputs(x, enc_h, enc_c, Wih, Whh, bih, bhh, d, q, S_):
    """Build the in_map for core (direction d in {0 fwd, 1 bwd}, quarter q)."""
    bsl = slice(q * BL, (q + 1) * BL)
    off = 0 if d == 0 else H
    xs = x[bsl, :S_]
    if d == 1:
        xs = xs[:, ::-1]
    m = {}
    import ml_dtypes

    bf = ml_dtypes.bfloat16
    m["xT"] = np.ascontiguousarray(
        xs.transpose(2, 1, 0).reshape(D, S_ * BL)
    ).astype(bf)
    for l in range(L):
        wp_ih = Wih[l][GATE_PERM]
        wp_hh = Whh[l][GATE_PERM]
        bp = (bih[l] + bhh[l])[GATE_PERM]
        m[f"wih{l}T"] = np.ascontiguousarray(wp_ih.T).astype(bf)
        m[f"whh{l}T"] = np.ascontiguousarray(wp_hh.T).astype(bf)
        m[f"bias{l}"] = np.ascontiguousarray(bp[None, :]).astype(bf)
        hvec = enc_h[l, bsl, off : off + H]  # [BL, H]
        m[f"ht0_{l}"] = np.ascontiguousarray(
            hvec.T.reshape(4, 128, BL).transpose(1, 0, 2).reshape(128, 4 * BL)
        ).astype(bf)
        m[f"c0_{l}"] = np.ascontiguousarray(
            enc_c[l, bsl, off : off + H], dtype=np.float32
        )
    m["ident8"] = np.eye(BL, dtype=np.float32)
    m["ident8b"] = np.eye(BL).astype(bf)
    return m


_CACHE = {}
LAST_EXEC_NS = None


def _get_program(S_):
    if S_ not in _CACHE:
        _CACHE[S_] = build_program(S_)
    return _CACHE[S_]


def kernel(x, enc_h, enc_c, Wih_f, Whh_f, bih_f, bhh_f, Wih_b, Whh_b, bih_b, bhh_b):
    from concourse.bass_utils import run_bass_kernel_spmd

    x = np.asarray(x, dtype=np.float32)
    enc_h = np.asarray(enc_h, dtype=np.float32)
    enc_c = np.asarray(enc_c, dtype=np.float32)
    Ws = {
        0: (np.asarray(Wih_f, np.float32), np.asarray(Whh_f, np.float32),
            np.asarray(bih_f, np.float32), np.asarray(bhh_f, np.float32)),
        1: (np.asarray(Wih_b, np.float32), np.asarray(Whh_b, np.float32),
            np.asarray(bih_b, np.float32), np.asarray(bhh_b, np.float32)),
    }

    nc = _get_program(S)
    in_maps = []
    for cid in range(NCORES):
        d, q = cid // 4, cid % 4
        wih, whh, bi, bh = Ws[d]
        in_maps.append(
            _core_inputs(x, enc_h, enc_c, wih, whh, bi, bh, d, q, S)
        )
    import os

    want_trace = os.environ.get("KERNEL_TRACE", "0") == "1"
    res_obj = run_bass_kernel_spmd(
        nc, in_maps, list(range(NCORES)), trace=want_trace
    )
    res = res_obj.results
    global LAST_EXEC_NS, LAST_RESULTS
    LAST_RESULTS = res_obj
    if res_obj.exec_time_ns is not None:
        LAST_EXEC_NS = res_obj.exec_time_ns

    out = _assemble(res)
    return out


def _assemble(res):
    out = np.empty((B, S, 2 * H), dtype=np.float32)
    h = np.empty((L, B, 2 * H), dtype=np.float32)
    c = np.empty((L, B, 2 * H), dtype=np.float32)
    for cid in range(NCORES):
        d, q = cid // 4, cid % 4
        bsl = slice(q * BL, (q + 1) * BL)
        off = 0 if d == 0 else H
        r = res[cid]
        o1 = r["out1"].reshape(S, BL, H).transpose(1, 0, 2)  # [BL, S, H]
        if d == 1:
            o1 = o1[:, ::-1]
        out[bsl, :, off : off + H] = o1
        h[:, bsl, off : off + H] = r["hN"]
        c[:, bsl, off : off + H] = r["cN"]
    return out, h, c


def bench(inputs, iters=5, S_=S):
    """Time pure NEFF executions (compile + transfers excluded).

    Mirrors bass2jax.run_bass_via_pjrt's multi-core path with inputs
    pre-placed on device; returns (best_seconds, per_iter_list, results).
    """
    import time

    import jax
    import jax.numpy as jnp
    from jax.experimental.shard_map import shard_map
    from jax.sharding import Mesh, NamedSharding, PartitionSpec

    from concourse import bass2jax, mybir as mb

    nc = _get_program(S_)
    x = np.asarray(inputs["x"], np.float32)
    enc_h = np.asarray(inputs["enc_h"], np.float32)
    enc_c = np.asarray(inputs["enc_c"], np.float32)
    Ws = {
        0: tuple(np.asarray(inputs[k], np.float32)
                 for k in ("Wih_f", "Whh_f", "bih_f", "bhh_f")),
        1: tuple(np.asarray(inputs[k], np.float32)
                 for k in ("Wih_b", "Whh_b", "bih_b", "bhh_b")),
    }
    in_maps = []
    for cid in range(NCORES):
        d, q = cid // 4, cid % 4
        wih, whh, bi, bh = Ws[d]
        in_maps.append(_core_inputs(x, enc_h, enc_c, wih, whh, bi, bh, d, q, S_))

    bass2jax.install_neuronx_cc_hook()
    partition_name = (
        nc.partition_id_tensor.name if nc.partition_id_tensor else None
    )
    in_names, out_names, out_avals, zero_outs = [], [], [], []
    for alloc in nc.m.functions[0].allocations:
        if not isinstance(alloc, mb.MemoryLocationSet):
            continue
        name = alloc.memorylocations[0].name
        if alloc.kind == "ExternalInput":
            if name != partition_name:
                in_names.append(name)
        elif alloc.kind == "ExternalOutput":
            out_names.append(name)
            shape = tuple(alloc.tensor_shape)
            dtype = mb.dt.np(alloc.dtype)
            out_avals.append(jax.core.ShapedArray(shape, dtype))
            zero_outs.append(np.zeros(shape, dtype))
    n_params = len(in_names)
    n_outs = len(out_avals)
    all_in_names = list(in_names) + out_names
    if partition_name is not None:
        all_in_names.append(partition_name)
    donate = tuple(range(n_params, n_params + n_outs))

    def _body(*args):
        operands = list(args)
        if partition_name is not None:
            operands.append(bass2jax.partition_id_tensor())
        outs = bass2jax._bass_exec_p.bind(
            *operands,
            out_avals=tuple(out_avals),
            in_names=tuple(all_in_names),
            out_names=tuple(out_names),
            lowering_input_output_aliases=(),
            sim_require_finite=True,
            sim_require_nnan=True,
            nc=nc,
        )
        return tuple(outs)

    devices = jax.devices()[:NCORES]
    mesh = Mesh(np.asarray(devices), ("core",))
    in_specs = (PartitionSpec("core"),) * (n_params + n_outs)
    out_specs = (PartitionSpec("core"),) * n_outs
    sharded = jax.jit(
        shard_map(_body, mesh=mesh, in_specs=in_specs,
                  out_specs=out_specs, check_rep=False),
        donate_argnums=donate, keep_unused=True,
    )
    shard = NamedSharding(mesh, PartitionSpec("core"))
    concat_in = [
        jax.device_put(
            np.concatenate([in_maps[c][n] for c in range(NCORES)], axis=0),
            shard,
        )
        for n in in_names
    ]
    for a in concat_in:
        a.block_until_ready()

    def fresh_zeros():
        return [
            jax.device_put(
                np.zeros((NCORES * z.shape[0], *z.shape[1:]), z.dtype), shard
            )
            for z in zero_outs
        ]

    # warmup (compiles)
    zs = fresh_zeros()
    [a.block_until_ready() for a in zs]
    out_arrs = sharded(*concat_in, *zs)
    jax.block_until_ready(out_arrs)

    times = []
    for _ in range(iters):
        zs = fresh_zeros()
        [a.block_until_ready() for a in zs]
        t0 = time.perf_counter()
        out_arrs = sharded(*concat_in, *zs)
        jax.block_until_ready(out_arrs)
        times.append(time.perf_counter() - t0)

    res = [
        {
            name: np.asarray(out_arrs[i]).reshape(NCORES, *out_avals[i].shape)[c]
            for i, name in enumerate(out_names)
        }
        for c in range(NCORES)
    ]
    return min(times), times, (_assemble(res) if S_ == S else res)


# revision 18
# speedup vs baseline: 1.6219x; 1.3575x over previous
# Bidirectional 2-layer LSTM decoder on 8 Trainium2 NeuronCores.
#
# Decomposition: the network factors into independent (batch, direction)
# chains — directions only concatenate at the output, and layer 1 of a
# direction consumes only that direction's layer-0 output. So the 8 cores
# run one uniform SPMD program: core = (direction, batch-quarter), with
# the direction realized purely through per-core data (time-reversed x and
# that direction's weights).
#
# Per core (B_local=8, S=512, H=512):
#   GEMM0:  G0 = x @ Wih0^T + bias     (big matmul, written to DRAM)
#   REC0:   512-step LSTM recurrence, layer 0
#   GEMM1:  G1 = out0 @ Wih1^T + bias
#   REC1:   512-step recurrence, layer 1 -> out, final h/c
#
# Recurrence step (batch-major, gates column order [g|i|f|o]):
#   gates_psum  = I8.T @ G[t]          (identity matmul folds the
#                                       precomputed input term into PSUM)
#   gates_psum += h_{t-1} @ Whh^T      (h^T is the tiny stationary operand;
#                                       the weight matrix streams, which is
#                                       what the PE does at full rate)
#   ACT: tanh(g), sigmoid(i,f), sigmoid(o), tanh(c')
#   DVE: c' = sf*c + si*tg ; h' = so*tanh(c')
#   PE:  4x transpose h' -> h'^T       (stationary operand for step t+1)

import sys

import numpy as np

for _p in ("/opt/trn_rl_repo", "/root/.axon_site/_ro/trn_rl_repo"):
    if _p not in sys.path:
        sys.path.append(_p)

import concourse.bass as bass  # noqa: E402
import concourse.mybir as mybir  # noqa: E402
import concourse.tile as tile  # noqa: E402
from concourse import bacc  # noqa: E402

F32 = mybir.dt.float32
BF16 = mybir.dt.bfloat16
AF = mybir.ActivationFunctionType

H = 512
L = 2
B = 32
S = 512
D = 512
NCORES = 8
BL = B // (NCORES // 2)  # 8: batch rows per core (2 dirs x 4 quarters)
G4 = 4 * H  # 2048 gate columns

# reorder torch gate rows (i,f,g,o) -> (g,i,f,o) so tanh(g) input is ready
# first in the matmul stream and sigmoid(i,f) reads one contiguous slab
GATE_PERM = np.r_[2 * H : 3 * H, 0:H, H : 2 * H, 3 * H : 4 * H]
SL_G = slice(0, H)
SL_IF = slice(H, 3 * H)
SL_I = slice(H, 2 * H)
SL_F = slice(2 * H, 3 * H)
SL_O = slice(3 * H, 4 * H)


def _emit_gemm(nc, pools, S_, lhsT_src, rhs_sb, bias_sb, ones1, G_dram):
    """G_dram[m*128:(m+1)*128, :] = lhsT_m.T @ rhs (+ ones1.T @ bias row).

    lhsT_src(m) -> list of 4 [128,128] APs (K-chunks of the stationary
    operand for output row-tile m). rhs_sb is [128, 4, G4] in SBUF.
    """
    n_m = (S_ * BL) // 128
    for m in range(n_m):
        lhsT = lhsT_src(m)
        gout = pools["gsb"].tile([128, G4], BF16, tag="gsb")
        for n in range(4):
            ps = pools["psum_g"].tile([128, H], F32, tag="ps_gem", name="psg")
            nc.tensor.matmul(
                ps[:],
                ones1[:],
                bias_sb[:, n * H : (n + 1) * H],
                start=True,
                stop=False,
            )
            for k in range(4):
                nc.tensor.matmul(
                    ps[:],
                    lhsT[k],
                    rhs_sb[:, k, n * H : (n + 1) * H],
                    start=False,
                    stop=(k == 3),
                )
            dst = gout[:, n * H : (n + 1) * H]
            if n % 2 == 0:
                nc.scalar.copy(dst, ps[:])
            else:
                nc.vector.tensor_copy(dst, ps[:])
        nc.gpsimd.dma_start(G_dram[m * 128 : (m + 1) * 128, :], gout[:])


def _emit_recurrence(
    nc, tc, cid, S_, layer, whh_sb, G_dram, ht_init, c_init_dram, ident8,
    ident8b, out0T_dram, out1_dram, hN_dram, cN_dram, ctx,
):
    """One 512-step LSTM chain with chain-private pools so two chains can
    interleave. Gates are computed in two [8,1024] PSUM halves (bank budget):
    half A = [g|i], half B = [f|o]."""
    HH = 2 * H
    ps = ctx.enter_context(tc.tile_pool(name=f"ps{cid}", bufs=2, space="PSUM"))
    tpsp = ctx.enter_context(tc.tile_pool(name=f"tps{cid}", bufs=1, space="PSUM"))
    gqp = ctx.enter_context(tc.tile_pool(name=f"gq{cid}", bufs=3))
    actp = ctx.enter_context(tc.tile_pool(name=f"act{cid}", bufs=2))
    dvep = ctx.enter_context(tc.tile_pool(name=f"dve{cid}", bufs=2))
    cp = ctx.enter_context(tc.tile_pool(name=f"c{cid}", bufs=3))
    hp = ctx.enter_context(tc.tile_pool(name=f"h{cid}", bufs=3))
    htp = ctx.enter_context(tc.tile_pool(name=f"ht{cid}", bufs=3))
    blkp = ctx.enter_context(tc.tile_pool(name=f"blk{cid}", bufs=3))

    c_init = cp.tile([BL, H], F32, tag="c", name=f"c_init{cid}")
    nc.sync.dma_start(c_init[:], c_init_dram[:])

    blk = None
    ht_prev = None
    prev_blk, prev_off = None, 0
    c_prev = c_init
    for t in range(S_):
        g_sb = gqp.tile([BL, G4], BF16, tag="gq", name=f"g_sb{cid}")
        nc.sync.dma_start(g_sb[:], G_dram[t * BL : (t + 1) * BL, :])

        def lhsT_k(k):
            if t == 0:
                return ht_init[:, k * BL : (k + 1) * BL]
            if layer == 0:
                return prev_blk[:, k, prev_off * BL : (prev_off + 1) * BL]
            return ht_prev[:, k * BL : (k + 1) * BL]

        # four [8,512] gate quarters (order g,i,f,o), each its own PSUM bank
        qt = []
        for q in range(4):
            gq_ps = ps.tile([BL, H], F32, tag="gates", name=f"gates{cid}")
            nc.tensor.matmul(
                gq_ps[:],
                ident8b[:],
                g_sb[:, q * H : (q + 1) * H],
                start=True,
                stop=False,
            )
            for k in range(4):
                nc.tensor.matmul(
                    gq_ps[:],
                    lhsT_k(k),
                    whh_sb[:, k, q * H : (q + 1) * H],
                    start=False,
                    stop=(k == 3),
                )
            qt.append(gq_ps)
        tg = actp.tile([BL, H], F32, tag="tg", name=f"tg{cid}")
        nc.scalar.activation(tg[:], qt[0][:], AF.Tanh)
        si = actp.tile([BL, H], F32, tag="si", name=f"si{cid}")
        nc.scalar.activation(si[:], qt[1][:], AF.Sigmoid)
        sf = actp.tile([BL, H], F32, tag="sf", name=f"sf{cid}")
        nc.scalar.activation(sf[:], qt[2][:], AF.Sigmoid)
        so = actp.tile([BL, H], F32, tag="so", name=f"so{cid}")
        nc.scalar.activation(so[:], qt[3][:], AF.Sigmoid)

        tmp2 = dvep.tile([BL, H], F32, tag="tmp2", name=f"tmp2_{cid}")
        nc.vector.tensor_mul(tmp2[:], si[:], tg[:])
        tmp1 = dvep.tile([BL, H], F32, tag="tmp1", name=f"tmp1_{cid}")
        nc.vector.tensor_mul(tmp1[:], sf[:], c_prev[:])
        c_new = cp.tile([BL, H], F32, tag="c", name=f"c{cid}")
        nc.vector.tensor_add(c_new[:], tmp1[:], tmp2[:])
        tc_t = actp.tile([BL, H], F32, tag="tc", name=f"tc{cid}")
        nc.scalar.activation(tc_t[:], c_new[:], AF.Tanh)
        h_new = hp.tile([BL, H], F32, tag="h", name=f"h{cid}")
        nc.vector.tensor_mul(h_new[:], so[:], tc_t[:])

        tps = tpsp.tile([128, 4 * BL], F32, tag="tps", name=f"tps{cid}")
        for k in range(4):
            nc.tensor.transpose(
                tps[:, k * BL : (k + 1) * BL],
                h_new[:, k * 128 : (k + 1) * 128],
                ident8[:],
            )
        if layer == 0:
            off = t % 16
            if off == 0:
                blk = blkp.tile([128, 4, 16 * BL], BF16, tag="blk", name="blk")
            dst = blk[:, :, off * BL : (off + 1) * BL]
            nc.scalar.copy(dst, tps[:].rearrange("p (k b) -> p k b", b=BL))
            if off == 15:
                m = t // 16
                nblk = 128 * 4 * 16 * BL
                nc.gpsimd.dma_start(
                    out0T_dram[m * nblk : (m + 1) * nblk]
                    .rearrange("(p k b) -> p k b", p=128, k=4),
                    blk[:],
                )
            prev_blk, prev_off = blk, off
        else:
            ht_new = htp.tile([128, 4 * BL], BF16, tag="ht", name=f"ht{cid}")
            nc.scalar.copy(ht_new[:], tps[:])
            ht_prev = ht_new
            nc.gpsimd.dma_start(out1_dram[t * BL : (t + 1) * BL, :], h_new[:])

        if t == S_ - 1:
            nc.gpsimd.dma_start(hN_dram[layer], h_new[:])
            nc.gpsimd.dma_start(cN_dram[layer], c_new[:])
        c_prev = c_new


def build_program(S_=S, debug=False):
    nc = bacc.Bacc(
        "TRN2",
        target_bir_lowering=False,
        debug=debug,
        num_devices=NCORES,
    )
    MT = S_ * BL  # GEMM output rows

    # --- I/O -------------------------------------------------------------
    xT = nc.dram_tensor("xT", [D, MT], BF16, kind="ExternalInput")
    wihT = [
        nc.dram_tensor(f"wih{l}T", [D, G4], BF16, kind="ExternalInput")
        for l in range(L)
    ]
    whhT = [
        nc.dram_tensor(f"whh{l}T", [H, G4], BF16, kind="ExternalInput")
        for l in range(L)
    ]
    bias = [
        nc.dram_tensor(f"bias{l}", [1, G4], BF16, kind="ExternalInput")
        for l in range(L)
    ]
    ht0 = [
        nc.dram_tensor(f"ht0_{l}", [128, 4 * BL], BF16, kind="ExternalInput")
        for l in range(L)
    ]
    c0 = [
        nc.dram_tensor(f"c0_{l}", [BL, H], F32, kind="ExternalInput")
        for l in range(L)
    ]
    ident_in = nc.dram_tensor("ident8", [BL, BL], F32, kind="ExternalInput")
    identb_in = nc.dram_tensor("ident8b", [BL, BL], BF16, kind="ExternalInput")

    out1 = nc.dram_tensor("out1", [MT, H], F32, kind="ExternalOutput")
    hN = nc.dram_tensor("hN", [L, BL, H], F32, kind="ExternalOutput")
    cN = nc.dram_tensor("cN", [L, BL, H], F32, kind="ExternalOutput")

    G0_dram = nc.dram_tensor("G0_i", [MT, G4], BF16)
    G1_dram = nc.dram_tensor("G1_i", [MT, G4], BF16)
    out0T_dram = nc.dram_tensor("out0T_i", [MT * D], BF16)

    with tile.TileContext(nc) as tc:
        from contextlib import ExitStack

        ctx = ExitStack()
        with (
            tc.tile_pool(name="const", bufs=1) as constp,
            tc.tile_pool(name="psum_g", bufs=2, space="PSUM") as psumgp,
            tc.tile_pool(name="w", bufs=1) as wp,
            tc.tile_pool(name="gsb", bufs=1) as gsbp,
            tc.tile_pool(name="lhsT_m", bufs=3) as lhsmp,
            ctx,
        ):
            pools = {"psum_g": psumgp, "gsb": gsbp}
            ident8 = constp.tile([BL, BL], F32, tag="ident")
            nc.sync.dma_start(ident8[:], ident_in[:])
            ident8b = constp.tile([BL, BL], BF16, tag="identb")
            nc.sync.dma_start(ident8b[:], identb_in[:])
            ones1 = constp.tile([1, 128], BF16, tag="ones1")
            nc.vector.memset(ones1[:], 1.0)
            bias_sb = [constp.tile([1, G4], BF16, tag=f"bias{l}", name=f"bias_sb{l}") for l in range(L)]
            ht_init = [constp.tile([128, 4 * BL], BF16, tag=f"ht0_{l}", name=f"ht_init{l}") for l in range(L)]
            for l in range(L):
                nc.sync.dma_start(bias_sb[l][:], bias[l][:])
                nc.sync.dma_start(ht_init[l][:], ht0[l][:])

            def load_w(dram, kdim, tag):
                t = wp.tile([128, kdim // 128, G4], BF16, tag=tag, name=tag)
                nc.sync.dma_start(
                    t[:], dram[:].rearrange("(k p) n -> p k n", p=128)
                )
                return t

            # ---- phase 0: G0 = x @ Wih0^T + b0 --------------------------
            wih0_sb = load_w(wihT[0], D, "w_ih")

            def lhsT_x(m):
                lt = lhsmp.tile([128, 4, 128], BF16, tag="lhsm")
                nc.sync.dma_start(
                    lt[:],
                    xT[:, m * 128 : (m + 1) * 128].rearrange(
                        "(k p) m -> p k m", p=128
                    ),
                )
                return [lt[:, k, :] for k in range(4)]

            _emit_gemm(nc, pools, S_, lhsT_x, wih0_sb, bias_sb[0], ones1, G0_dram)

            # ---- phase 1: layer-0 recurrence ----------------------------
            whh0_sb = load_w(whhT[0], H, "w_hh0")
            _emit_recurrence(
                nc, tc, 0, S_, 0, whh0_sb, G0_dram, ht_init[0], c0[0],
                ident8, ident8b, out0T_dram, None, hN, cN, ctx,
            )

            # ---- phase 2: G1 = out0 @ Wih1^T + b1 -----------------------
            wih1_sb = load_w(wihT[1], D, "w_ih")

            def lhsT_o(m):
                lt = lhsmp.tile([128, 4, 128], BF16, tag="lhsm")
                nc.sync.dma_start(
                    lt[:],
                    out0T_dram[m * 128 * 512 : (m + 1) * 128 * 512].rearrange(
                        "(p k m) -> p k m", p=128, k=4
                    ),
                )
                return [lt[:, k, :] for k in range(4)]

            _emit_gemm(nc, pools, S_, lhsT_o, wih1_sb, bias_sb[1], ones1, G1_dram)

            # ---- phase 3: layer-1 recurrence ----------------------------
            whh1_sb = load_w(whhT[1], H, "w_hh1")
            _emit_recurrence(
                nc, tc, 1, S_, 1, whh1_sb, G1_dram, ht_init[1], c0[1],
                ident8, ident8b, None, out1, hN, cN, ctx,
            )

    nc.compile()
    return nc


# ---------------------------------------------------------------------------
# host side
# ---------------------------------------------------------------------------

def _core_inputs(x, enc_h, enc_c, Wih, Whh, bih, bhh, d, q, S_):
    """Build the in_map for core (direction d in {0 fwd, 1 bwd}, quarter q)."""
    bsl = slice(q * BL, (q + 1) * BL)
    off = 0 if d == 0 else H
    xs = x[bsl, :S_]
    if d == 1:
        xs = xs[:, ::-1]
    m = {}
    import ml_dtypes

    bf = ml_dtypes.bfloat16
    m["xT"] = np.ascontiguousarray(
        xs.transpose(2, 1, 0).reshape(D, S_ * BL)
    ).astype(bf)
    for l in range(L):
        wp_ih = Wih[l][GATE_PERM]
        wp_hh = Whh[l][GATE_PERM]
        bp = (bih[l] + bhh[l])[GATE_PERM]
        m[f"wih{l}T"] = np.ascontiguousarray(wp_ih.T).astype(bf)
        m[f"whh{l}T"] = np.ascontiguousarray(wp_hh.T).astype(bf)
        m[f"bias{l}"] = np.ascontiguousarray(bp[None, :]).astype(bf)
        hvec = enc_h[l, bsl, off : off + H]  # [BL, H]
        m[f"ht0_{l}"] = np.ascontiguousarray(
            hvec.T.reshape(4, 128, BL).transpose(1, 0, 2).reshape(128, 4 * BL)
        ).astype(bf)
        m[f"c0_{l}"] = np.ascontiguousarray(
            enc_c[l, bsl, off : off + H], dtype=np.float32
        )
    m["ident8"] = np.eye(BL, dtype=np.float32)
    m["ident8b"] = np.eye(BL).astype(bf)
    return m


_CACHE = {}
LAST_EXEC_NS = None


def _get_program(S_):
    if S_ not in _CACHE:
        _CACHE[S_] = build_program(S_)
    return _CACHE[S_]


def kernel(x, enc_h, enc_c, Wih_f, Whh_f, bih_f, bhh_f, Wih_b, Whh_b, bih_b, bhh_b):
    from concourse.bass_utils import run_bass_kernel_spmd

    x = np.asarray(x, dtype=np.float32)
    enc_h = np.asarray(enc_h, dtype=np.float32)
    enc_c = np.asarray(enc_c, dtype=np.float32)
    Ws = {
        0: (np.asarray(Wih_f, np.float32), np.asarray(Whh_f, np.float32),
            np.asarray(bih_f, np.float32), np.asarray(bhh_f, np.float32)),
        1: (np.asarray(Wih_b, np.float32), np.asarray(Whh_b, np.float32),
            np.asarray(bih_b, np.float32), np.asarray(bhh_b, np.float32)),
    }

    nc = _get_program(S)
    in_maps = []
    for cid in range(NCORES):
        d, q = cid // 4, cid % 4
        wih, whh, bi, bh = Ws[d]
        in_maps.append(
            _core_inputs(x, enc_h, enc_c, wih, whh, bi, bh, d, q, S)
        )
    import os

    want_trace = os.environ.get("KERNEL_TRACE", "0") == "1"
    res_obj = run_bass_kernel_spmd(
        nc, in_maps, list(range(NCORES)), trace=want_trace
    )
    res = res_obj.results
    global LAST_EXEC_NS, LAST_RESULTS
    LAST_RESULTS = res_obj
    if res_obj.exec_time_ns is not None:
        LAST_EXEC_NS = res_obj.exec_time_ns

    out = _assemble(res)
    return out


def _assemble(res):
    out = np.empty((B, S, 2 * H), dtype=np.float32)
    h = np.empty((L, B, 2 * H), dtype=np.float32)
    c = np.empty((L, B, 2 * H), dtype=np.float32)
    for cid in range(NCORES):
        d, q = cid // 4, cid % 4
        bsl = slice(q * BL, (q + 1) * BL)
        off = 0 if d == 0 else H
        r = res[cid]
        o1 = r["out1"].reshape(S, BL, H).transpose(1, 0, 2)  # [BL, S, H]
        if d == 1:
            o1 = o1[:, ::-1]
        out[bsl, :, off : off + H] = o1
        h[:, bsl, off : off + H] = r["hN"]
        c[:, bsl, off : off + H] = r["cN"]
    return out, h, c


def bench(inputs, iters=5, S_=S):
    """Time pure NEFF executions (compile + transfers excluded).

    Mirrors bass2jax.run_bass_via_pjrt's multi-core path with inputs
    pre-placed on device; returns (best_seconds, per_iter_list, results).
    """
    import time

    import jax
    import jax.numpy as jnp
    from jax.experimental.shard_map import shard_map
    from jax.sharding import Mesh, NamedSharding, PartitionSpec

    from concourse import bass2jax, mybir as mb

    nc = _get_program(S_)
    x = np.asarray(inputs["x"], np.float32)
    enc_h = np.asarray(inputs["enc_h"], np.float32)
    enc_c = np.asarray(inputs["enc_c"], np.float32)
    Ws = {
        0: tuple(np.asarray(inputs[k], np.float32)
                 for k in ("Wih_f", "Whh_f", "bih_f", "bhh_f")),
        1: tuple(np.asarray(inputs[k], np.float32)
                 for k in ("Wih_b", "Whh_b", "bih_b", "bhh_b")),
    }
    in_maps = []
    for cid in range(NCORES):
        d, q = cid // 4, cid % 4
        wih, whh, bi, bh = Ws[d]
        in_maps.append(_core_inputs(x, enc_h, enc_c, wih, whh, bi, bh, d, q, S_))

    bass2jax.install_neuronx_cc_hook()
    partition_name = (
        nc.partition_id_tensor.name if nc.partition_id_tensor else None
    )
    in_names, out_names, out_avals, zero_outs = [], [], [], []
    for alloc in nc.m.functions[0].allocations:
        if not isinstance(alloc, mb.MemoryLocationSet):
            continue
        name = alloc.memorylocations[0].name
        if alloc.kind == "ExternalInput":
            if name != partition_name:
                in_names.append(name)
        elif alloc.kind == "ExternalOutput":
            out_names.append(name)
            shape = tuple(alloc.tensor_shape)
            dtype = mb.dt.np(alloc.dtype)
            out_avals.append(jax.core.ShapedArray(shape, dtype))
            zero_outs.append(np.zeros(shape, dtype))
    n_params = len(in_names)
    n_outs = len(out_avals)
    all_in_names = list(in_names) + out_names
    if partition_name is not None:
        all_in_names.append(partition_name)
    donate = tuple(range(n_params, n_params + n_outs))

    def _body(*args):
        operands = list(args)
        if partition_name is not None:
            operands.append(bass2jax.partition_id_tensor())
        outs = bass2jax._bass_exec_p.bind(
            *operands,
            out_avals=tuple(out_avals),
            in_names=tuple(all_in_names),
            out_names=tuple(out_names),
            lowering_input_output_aliases=(),
            sim_require_finite=True,
            sim_require_nnan=True,
            nc=nc,
        )
        return tuple(outs)

    devices = jax.devices()[:NCORES]
    mesh = Mesh(np.asarray(devices), ("core",))
    in_specs = (PartitionSpec("core"),) * (n_params + n_outs)
    out_specs = (PartitionSpec("core"),) * n_outs
    sharded = jax.jit(
        shard_map(_body, mesh=mesh, in_specs=in_specs,
                  out_specs=out_specs, check_rep=False),
        donate_argnums=donate, keep_unused=True,
    )
    shard = NamedSharding(mesh, PartitionSpec("core"))
    concat_in = [
        jax.device_put(
            np.concatenate([in_maps[c][n] for c in range(NCORES)], axis=0),
            shard,
        )
        for n in in_names
    ]
    for a in concat_in:
        a.block_until_ready()

    def fresh_zeros():
        return [
            jax.device_put(
                np.zeros((NCORES * z.shape[0], *z.shape[1:]), z.dtype), shard
            )
            for z in zero_outs
        ]

    # warmup (compiles)
    zs = fresh_zeros()
    [a.block_until_ready() for a in zs]
    out_arrs = sharded(*concat_in, *zs)
    jax.block_until_ready(out_arrs)

    times = []
    for _ in range(iters):
        zs = fresh_zeros()
        [a.block_until_ready() for a in zs]
        t0 = time.perf_counter()
        out_arrs = sharded(*concat_in, *zs)
        jax.block_until_ready(out_arrs)
        times.append(time.perf_counter() - t0)

    res = [
        {
            name: np.asarray(out_arrs[i]).reshape(NCORES, *out_avals[i].shape)[c]
            for i, name in enumerate(out_names)
        }
        for c in range(NCORES)
    ]
    return min(times), times, (_assemble(res) if S_ == S else res)


# revision 19
# speedup vs baseline: 1.6315x; 1.0060x over previous
# Bidirectional 2-layer LSTM decoder on 8 Trainium2 NeuronCores.
#
# Decomposition: the network factors into independent (batch, direction)
# chains — directions only concatenate at the output, and layer 1 of a
# direction consumes only that direction's layer-0 output. So the 8 cores
# run one uniform SPMD program: core = (direction, batch-quarter), with
# the direction realized purely through per-core data (time-reversed x and
# that direction's weights).
#
# Per core (B_local=8, S=512, H=512):
#   GEMM0:  G0 = x @ Wih0^T + bias     (big matmul, written to DRAM)
#   REC0:   512-step LSTM recurrence, layer 0
#   GEMM1:  G1 = out0 @ Wih1^T + bias
#   REC1:   512-step recurrence, layer 1 -> out, final h/c
#
# Recurrence step (batch-major, gates column order [g|i|f|o]):
#   gates_psum  = I8.T @ G[t]          (identity matmul folds the
#                                       precomputed input term into PSUM)
#   gates_psum += h_{t-1} @ Whh^T      (h^T is the tiny stationary operand;
#                                       the weight matrix streams, which is
#                                       what the PE does at full rate)
#   ACT: tanh(g), sigmoid(i,f), sigmoid(o), tanh(c')
#   DVE: c' = sf*c + si*tg ; h' = so*tanh(c')
#   PE:  4x transpose h' -> h'^T       (stationary operand for step t+1)

import sys

import numpy as np

for _p in ("/opt/trn_rl_repo", "/root/.axon_site/_ro/trn_rl_repo"):
    if _p not in sys.path:
        sys.path.append(_p)

import concourse.bass as bass  # noqa: E402
import concourse.mybir as mybir  # noqa: E402
import concourse.tile as tile  # noqa: E402
from concourse import bacc  # noqa: E402

F32 = mybir.dt.float32
BF16 = mybir.dt.bfloat16
AF = mybir.ActivationFunctionType

H = 512
L = 2
B = 32
S = 512
D = 512
NCORES = 8
BL = B // (NCORES // 2)  # 8: batch rows per core (2 dirs x 4 quarters)
G4 = 4 * H  # 2048 gate columns

# reorder torch gate rows (i,f,g,o) -> (g,i,f,o) so tanh(g) input is ready
# first in the matmul stream and sigmoid(i,f) reads one contiguous slab
GATE_PERM = np.r_[2 * H : 3 * H, 0:H, H : 2 * H, 3 * H : 4 * H]
SL_G = slice(0, H)
SL_IF = slice(H, 3 * H)
SL_I = slice(H, 2 * H)
SL_F = slice(2 * H, 3 * H)
SL_O = slice(3 * H, 4 * H)


def _emit_gemm(nc, pools, S_, lhsT_src, rhs_sb, bias_sb, ones1, G_dram):
    """G_dram[m*128:(m+1)*128, :] = lhsT_m.T @ rhs (+ ones1.T @ bias row).

    lhsT_src(m) -> list of 4 [128,128] APs (K-chunks of the stationary
    operand for output row-tile m). rhs_sb is [128, 4, G4] in SBUF.
    """
    n_m = (S_ * BL) // 128
    for m in range(n_m):
        lhsT = lhsT_src(m)
        gout = pools["gsb"].tile([128, G4], BF16, tag="gsb")
        for n in range(4):
            ps = pools["psum_g"].tile([128, H], F32, tag="ps_gem", name="psg")
            nc.tensor.matmul(
                ps[:],
                ones1[:],
                bias_sb[:, n * H : (n + 1) * H],
                start=True,
                stop=False,
            )
            for k in range(4):
                nc.tensor.matmul(
                    ps[:],
                    lhsT[k],
                    rhs_sb[:, k, n * H : (n + 1) * H],
                    start=False,
                    stop=(k == 3),
                )
            dst = gout[:, n * H : (n + 1) * H]
            if n % 2 == 0:
                nc.scalar.copy(dst, ps[:])
            else:
                nc.vector.tensor_copy(dst, ps[:])
        nc.sync.dma_start(G_dram[m * 128 : (m + 1) * 128, :], gout[:])


def _emit_recurrence(
    nc, tc, cid, S_, layer, whh_sb, G_dram, ht_init, c_init_dram, ident8,
    ident8b, out0T_dram, out1_dram, hN_dram, cN_dram, ctx,
):
    """One 512-step LSTM chain with chain-private pools so two chains can
    interleave. Gates are computed in two [8,1024] PSUM halves (bank budget):
    half A = [g|i], half B = [f|o]."""
    HH = 2 * H
    ps = ctx.enter_context(tc.tile_pool(name=f"ps{cid}", bufs=2, space="PSUM"))
    tpsp = ctx.enter_context(tc.tile_pool(name=f"tps{cid}", bufs=1, space="PSUM"))
    gqp = ctx.enter_context(tc.tile_pool(name=f"gq{cid}", bufs=4))
    actp = ctx.enter_context(tc.tile_pool(name=f"act{cid}", bufs=2))
    dvep = ctx.enter_context(tc.tile_pool(name=f"dve{cid}", bufs=2))
    cp = ctx.enter_context(tc.tile_pool(name=f"c{cid}", bufs=3))
    hp = ctx.enter_context(tc.tile_pool(name=f"h{cid}", bufs=3))
    htp = ctx.enter_context(tc.tile_pool(name=f"ht{cid}", bufs=3))
    blkp = ctx.enter_context(tc.tile_pool(name=f"blk{cid}", bufs=3))

    c_init = cp.tile([BL, H], F32, tag="c", name=f"c_init{cid}")
    nc.sync.dma_start(c_init[:], c_init_dram[:])

    blk = None
    ht_prev = None
    prev_blk, prev_off = None, 0
    c_prev = c_init
    for t in range(S_):
        g_sb = gqp.tile([BL, G4], BF16, tag="gq", name=f"g_sb{cid}")
        nc.sync.dma_start(g_sb[:], G_dram[t * BL : (t + 1) * BL, :])

        def lhsT_k(k):
            if t == 0:
                return ht_init[:, k * BL : (k + 1) * BL]
            if layer == 0:
                return prev_blk[:, k, prev_off * BL : (prev_off + 1) * BL]
            return ht_prev[:, k * BL : (k + 1) * BL]

        # four [8,512] gate quarters (order g,i,f,o), each its own PSUM bank
        qt = []
        for q in range(4):
            gq_ps = ps.tile([BL, H], F32, tag="gates", name=f"gates{cid}")
            nc.tensor.matmul(
                gq_ps[:],
                ident8b[:],
                g_sb[:, q * H : (q + 1) * H],
                start=True,
                stop=False,
            )
            for k in range(4):
                nc.tensor.matmul(
                    gq_ps[:],
                    lhsT_k(k),
                    whh_sb[:, k, q * H : (q + 1) * H],
                    start=False,
                    stop=(k == 3),
                )
            qt.append(gq_ps)
        tg = actp.tile([BL, H], F32, tag="tg", name=f"tg{cid}")
        nc.scalar.activation(tg[:], qt[0][:], AF.Tanh)
        si = actp.tile([BL, H], F32, tag="si", name=f"si{cid}")
        nc.scalar.activation(si[:], qt[1][:], AF.Sigmoid)
        sf = actp.tile([BL, H], F32, tag="sf", name=f"sf{cid}")
        nc.scalar.activation(sf[:], qt[2][:], AF.Sigmoid)
        so = actp.tile([BL, H], F32, tag="so", name=f"so{cid}")
        nc.scalar.activation(so[:], qt[3][:], AF.Sigmoid)

        tmp2 = dvep.tile([BL, H], F32, tag="tmp2", name=f"tmp2_{cid}")
        nc.vector.tensor_mul(tmp2[:], si[:], tg[:])
        tmp1 = dvep.tile([BL, H], F32, tag="tmp1", name=f"tmp1_{cid}")
        nc.vector.tensor_mul(tmp1[:], sf[:], c_prev[:])
        c_new = cp.tile([BL, H], F32, tag="c", name=f"c{cid}")
        nc.vector.tensor_add(c_new[:], tmp1[:], tmp2[:])
        tc_t = actp.tile([BL, H], F32, tag="tc", name=f"tc{cid}")
        nc.scalar.activation(tc_t[:], c_new[:], AF.Tanh)
        h_new = hp.tile([BL, H], F32, tag="h", name=f"h{cid}")
        nc.vector.tensor_mul(h_new[:], so[:], tc_t[:])

        tps = tpsp.tile([128, 4 * BL], F32, tag="tps", name=f"tps{cid}")
        for k in range(4):
            nc.tensor.transpose(
                tps[:, k * BL : (k + 1) * BL],
                h_new[:, k * 128 : (k + 1) * 128],
                ident8[:],
            )
        if layer == 0:
            off = t % 16
            if off == 0:
                blk = blkp.tile([128, 4, 16 * BL], BF16, tag="blk", name="blk")
            dst = blk[:, :, off * BL : (off + 1) * BL]
            nc.scalar.copy(dst, tps[:].rearrange("p (k b) -> p k b", b=BL))
            if off == 15:
                m = t // 16
                nblk = 128 * 4 * 16 * BL
                nc.gpsimd.dma_start(
                    out0T_dram[m * nblk : (m + 1) * nblk]
                    .rearrange("(p k b) -> p k b", p=128, k=4),
                    blk[:],
                )
            prev_blk, prev_off = blk, off
        else:
            ht_new = htp.tile([128, 4 * BL], BF16, tag="ht", name=f"ht{cid}")
            nc.scalar.copy(ht_new[:], tps[:])
            ht_prev = ht_new
            nc.sync.dma_start(out1_dram[t * BL : (t + 1) * BL, :], h_new[:])

        if t == S_ - 1:
            nc.sync.dma_start(hN_dram[layer], h_new[:])
            nc.sync.dma_start(cN_dram[layer], c_new[:])
        c_prev = c_new


def build_program(S_=S, debug=False):
    nc = bacc.Bacc(
        "TRN2",
        target_bir_lowering=False,
        debug=debug,
        num_devices=NCORES,
    )
    MT = S_ * BL  # GEMM output rows

    # --- I/O -------------------------------------------------------------
    xT = nc.dram_tensor("xT", [D, MT], BF16, kind="ExternalInput")
    wihT = [
        nc.dram_tensor(f"wih{l}T", [D, G4], BF16, kind="ExternalInput")
        for l in range(L)
    ]
    whhT = [
        nc.dram_tensor(f"whh{l}T", [H, G4], BF16, kind="ExternalInput")
        for l in range(L)
    ]
    bias = [
        nc.dram_tensor(f"bias{l}", [1, G4], BF16, kind="ExternalInput")
        for l in range(L)
    ]
    ht0 = [
        nc.dram_tensor(f"ht0_{l}", [128, 4 * BL], BF16, kind="ExternalInput")
        for l in range(L)
    ]
    c0 = [
        nc.dram_tensor(f"c0_{l}", [BL, H], F32, kind="ExternalInput")
        for l in range(L)
    ]
    ident_in = nc.dram_tensor("ident8", [BL, BL], F32, kind="ExternalInput")
    identb_in = nc.dram_tensor("ident8b", [BL, BL], BF16, kind="ExternalInput")

    out1 = nc.dram_tensor("out1", [MT, H], F32, kind="ExternalOutput")
    hN = nc.dram_tensor("hN", [L, BL, H], F32, kind="ExternalOutput")
    cN = nc.dram_tensor("cN", [L, BL, H], F32, kind="ExternalOutput")

    G0_dram = nc.dram_tensor("G0_i", [MT, G4], BF16)
    G1_dram = nc.dram_tensor("G1_i", [MT, G4], BF16)
    out0T_dram = nc.dram_tensor("out0T_i", [MT * D], BF16)

    with tile.TileContext(nc) as tc:
        from contextlib import ExitStack

        ctx = ExitStack()
        with (
            tc.tile_pool(name="const", bufs=1) as constp,
            tc.tile_pool(name="psum_g", bufs=2, space="PSUM") as psumgp,
            tc.tile_pool(name="w", bufs=1) as wp,
            tc.tile_pool(name="gsb", bufs=1) as gsbp,
            tc.tile_pool(name="lhsT_m", bufs=3) as lhsmp,
            ctx,
        ):
            pools = {"psum_g": psumgp, "gsb": gsbp}
            ident8 = constp.tile([BL, BL], F32, tag="ident")
            nc.sync.dma_start(ident8[:], ident_in[:])
            ident8b = constp.tile([BL, BL], BF16, tag="identb")
            nc.sync.dma_start(ident8b[:], identb_in[:])
            ones1 = constp.tile([1, 128], BF16, tag="ones1")
            nc.vector.memset(ones1[:], 1.0)
            bias_sb = [constp.tile([1, G4], BF16, tag=f"bias{l}", name=f"bias_sb{l}") for l in range(L)]
            ht_init = [constp.tile([128, 4 * BL], BF16, tag=f"ht0_{l}", name=f"ht_init{l}") for l in range(L)]
            for l in range(L):
                nc.sync.dma_start(bias_sb[l][:], bias[l][:])
                nc.sync.dma_start(ht_init[l][:], ht0[l][:])

            def load_w(dram, kdim, tag):
                t = wp.tile([128, kdim // 128, G4], BF16, tag=tag, name=tag)
                nc.sync.dma_start(
                    t[:], dram[:].rearrange("(k p) n -> p k n", p=128)
                )
                return t

            # ---- phase 0: G0 = x @ Wih0^T + b0 --------------------------
            wih0_sb = load_w(wihT[0], D, "w_ih")

            def lhsT_x(m):
                lt = lhsmp.tile([128, 4, 128], BF16, tag="lhsm")
                nc.sync.dma_start(
                    lt[:],
                    xT[:, m * 128 : (m + 1) * 128].rearrange(
                        "(k p) m -> p k m", p=128
                    ),
                )
                return [lt[:, k, :] for k in range(4)]

            _emit_gemm(nc, pools, S_, lhsT_x, wih0_sb, bias_sb[0], ones1, G0_dram)

            # ---- phase 1: layer-0 recurrence ----------------------------
            whh0_sb = load_w(whhT[0], H, "w_hh0")
            _emit_recurrence(
                nc, tc, 0, S_, 0, whh0_sb, G0_dram, ht_init[0], c0[0],
                ident8, ident8b, out0T_dram, None, hN, cN, ctx,
            )

            # ---- phase 2: G1 = out0 @ Wih1^T + b1 -----------------------
            wih1_sb = load_w(wihT[1], D, "w_ih")

            def lhsT_o(m):
                lt = lhsmp.tile([128, 4, 128], BF16, tag="lhsm")
                nc.sync.dma_start(
                    lt[:],
                    out0T_dram[m * 128 * 512 : (m + 1) * 128 * 512].rearrange(
                        "(p k m) -> p k m", p=128, k=4
                    ),
                )
                return [lt[:, k, :] for k in range(4)]

            _emit_gemm(nc, pools, S_, lhsT_o, wih1_sb, bias_sb[1], ones1, G1_dram)

            # ---- phase 3: layer-1 recurrence ----------------------------
            whh1_sb = load_w(whhT[1], H, "w_hh1")
            _emit_recurrence(
                nc, tc, 1, S_, 1, whh1_sb, G1_dram, ht_init[1], c0[1],
                ident8, ident8b, None, out1, hN, cN, ctx,
            )

    nc.compile()
    return nc


# ---------------------------------------------------------------------------
# host side
# ---------------------------------------------------------------------------

def _core_inputs(x, enc_h, enc_c, Wih, Whh, bih, bhh, d, q, S_):
    """Build the in_map for core (direction d in {0 fwd, 1 bwd}, quarter q)."""
    bsl = slice(q * BL, (q + 1) * BL)
    off = 0 if d == 0 else H
    xs = x[bsl, :S_]
    if d == 1:
        xs = xs[:, ::-1]
    m = {}
    import ml_dtypes

    bf = ml_dtypes.bfloat16
    m["xT"] = np.ascontiguousarray(
        xs.transpose(2, 1, 0).reshape(D, S_ * BL)
    ).astype(bf)
    for l in range(L):
        wp_ih = Wih[l][GATE_PERM]
        wp_hh = Whh[l][GATE_PERM]
        bp = (bih[l] + bhh[l])[GATE_PERM]
        m[f"wih{l}T"] = np.ascontiguousarray(wp_ih.T).astype(bf)
        m[f"whh{l}T"] = np.ascontiguousarray(wp_hh.T).astype(bf)
        m[f"bias{l}"] = np.ascontiguousarray(bp[None, :]).astype(bf)
        hvec = enc_h[l, bsl, off : off + H]  # [BL, H]
        m[f"ht0_{l}"] = np.ascontiguousarray(
            hvec.T.reshape(4, 128, BL).transpose(1, 0, 2).reshape(128, 4 * BL)
        ).astype(bf)
        m[f"c0_{l}"] = np.ascontiguousarray(
            enc_c[l, bsl, off : off + H], dtype=np.float32
        )
    m["ident8"] = np.eye(BL, dtype=np.float32)
    m["ident8b"] = np.eye(BL).astype(bf)
    return m


_CACHE = {}
LAST_EXEC_NS = None


def _get_program(S_):
    if S_ not in _CACHE:
        _CACHE[S_] = build_program(S_)
    return _CACHE[S_]


def kernel(x, enc_h, enc_c, Wih_f, Whh_f, bih_f, bhh_f, Wih_b, Whh_b, bih_b, bhh_b):
    from concourse.bass_utils import run_bass_kernel_spmd

    x = np.asarray(x, dtype=np.float32)
    enc_h = np.asarray(enc_h, dtype=np.float32)
    enc_c = np.asarray(enc_c, dtype=np.float32)
    Ws = {
        0: (np.asarray(Wih_f, np.float32), np.asarray(Whh_f, np.float32),
            np.asarray(bih_f, np.float32), np.asarray(bhh_f, np.float32)),
        1: (np.asarray(Wih_b, np.float32), np.asarray(Whh_b, np.float32),
            np.asarray(bih_b, np.float32), np.asarray(bhh_b, np.float32)),
    }

    nc = _get_program(S)
    in_maps = []
    for cid in range(NCORES):
        d, q = cid // 4, cid % 4
        wih, whh, bi, bh = Ws[d]
        in_maps.append(
            _core_inputs(x, enc_h, enc_c, wih, whh, bi, bh, d, q, S)
        )
    import os

    want_trace = os.environ.get("KERNEL_TRACE", "0") == "1"
    res_obj = run_bass_kernel_spmd(
        nc, in_maps, list(range(NCORES)), trace=want_trace
    )
    res = res_obj.results
    global LAST_EXEC_NS, LAST_RESULTS
    LAST_RESULTS = res_obj
    if res_obj.exec_time_ns is not None:
        LAST_EXEC_NS = res_obj.exec_time_ns

    out = _assemble(res)
    return out


def _assemble(res):
    out = np.empty((B, S, 2 * H), dtype=np.float32)
    h = np.empty((L, B, 2 * H), dtype=np.float32)
    c = np.empty((L, B, 2 * H), dtype=np.float32)
    for cid in range(NCORES):
        d, q = cid // 4, cid % 4
        bsl = slice(q * BL, (q + 1) * BL)
        off = 0 if d == 0 else H
        r = res[cid]
        o1 = r["out1"].reshape(S, BL, H).transpose(1, 0, 2)  # [BL, S, H]
        if d == 1:
            o1 = o1[:, ::-1]
        out[bsl, :, off : off + H] = o1
        h[:, bsl, off : off + H] = r["hN"]
        c[:, bsl, off : off + H] = r["cN"]
    return out, h, c


def bench(inputs, iters=5, S_=S):
    """Time pure NEFF executions (compile + transfers excluded).

    Mirrors bass2jax.run_bass_via_pjrt's multi-core path with inputs
    pre-placed on device; returns (best_seconds, per_iter_list, results).
    """
    import time

    import jax
    import jax.numpy as jnp
    from jax.experimental.shard_map import shard_map
    from jax.sharding import Mesh, NamedSharding, PartitionSpec

    from concourse import bass2jax, mybir as mb

    nc = _get_program(S_)
    x = np.asarray(inputs["x"], np.float32)
    enc_h = np.asarray(inputs["enc_h"], np.float32)
    enc_c = np.asarray(inputs["enc_c"], np.float32)
    Ws = {
        0: tuple(np.asarray(inputs[k], np.float32)
                 for k in ("Wih_f", "Whh_f", "bih_f", "bhh_f")),
        1: tuple(np.asarray(inputs[k], np.float32)
                 for k in ("Wih_b", "Whh_b", "bih_b", "bhh_b")),
    }
    in_maps = []
    for cid in range(NCORES):
        d, q = cid // 4, cid % 4
        wih, whh, bi, bh = Ws[d]
        in_maps.append(_core_inputs(x, enc_h, enc_c, wih, whh, bi, bh, d, q, S_))

    bass2jax.install_neuronx_cc_hook()
    partition_name = (
        nc.partition_id_tensor.name if nc.partition_id_tensor else None
    )
    in_names, out_names, out_avals, zero_outs = [], [], [], []
    for alloc in nc.m.functions[0].allocations:
        if not isinstance(alloc, mb.MemoryLocationSet):
            continue
        name = alloc.memorylocations[0].name
        if alloc.kind == "ExternalInput":
            if name != partition_name:
                in_names.append(name)
        elif alloc.kind == "ExternalOutput":
            out_names.append(name)
            shape = tuple(alloc.tensor_shape)
            dtype = mb.dt.np(alloc.dtype)
            out_avals.append(jax.core.ShapedArray(shape, dtype))
            zero_outs.append(np.zeros(shape, dtype))
    n_params = len(in_names)
    n_outs = len(out_avals)
    all_in_names = list(in_names) + out_names
    if partition_name is not None:
        all_in_names.append(partition_name)
    donate = tuple(range(n_params, n_params + n_outs))

    def _body(*args):
        operands = list(args)
        if partition_name is not None:
            operands.append(bass2jax.partition_id_tensor())
        outs = bass2jax._bass_exec_p.bind(
            *operands,
            out_avals=tuple(out_avals),
            in_names=tuple(all_in_names),
            out_names=tuple(out_names),
            lowering_input_output_aliases=(),
            sim_require_finite=True,
            sim_require_nnan=True,
            nc=nc,
        )
        return tuple(outs)

    devices = jax.devices()[:NCORES]
    mesh = Mesh(np.asarray(devices), ("core",))
    in_specs = (PartitionSpec("core"),) * (n_params + n_outs)
    out_specs = (PartitionSpec("core"),) * n_outs
    sharded = jax.jit(
        shard_map(_body, mesh=mesh, in_specs=in_specs,
                  out_specs=out_specs, check_rep=False),
        donate_argnums=donate, keep_unused=True,
    )
    shard = NamedSharding(mesh, PartitionSpec("core"))
    concat_in = [
        jax.device_put(
            np.concatenate([in_maps[c][n] for c in range(NCORES)], axis=0),
            shard,
        )
        for n in in_names
    ]
    for a in concat_in:
        a.block_until_ready()

    def fresh_zeros():
        return [
            jax.device_put(
                np.zeros((NCORES * z.shape[0], *z.shape[1:]), z.dtype), shard
            )
            for z in zero_outs
        ]

    # warmup (compiles)
    zs = fresh_zeros()
    [a.block_until_ready() for a in zs]
    out_arrs = sharded(*concat_in, *zs)
    jax.block_until_ready(out_arrs)

    times = []
    for _ in range(iters):
        zs = fresh_zeros()
        [a.block_until_ready() for a in zs]
        t0 = time.perf_counter()
        out_arrs = sharded(*concat_in, *zs)
        jax.block_until_ready(out_arrs)
        times.append(time.perf_counter() - t0)

    res = [
        {
            name: np.asarray(out_arrs[i]).reshape(NCORES, *out_avals[i].shape)[c]
            for i, name in enumerate(out_names)
        }
        for c in range(NCORES)
    ]
    return min(times), times, (_assemble(res) if S_ == S else res)
